# revision 1
# baseline (speedup 1.0000x reference)
"""BinarizedLinear Trainium2 kernel.

out = sign(x) @ sign(W).T + bias,  x:[8192,4096] W:[4096,4096] bias:[4096] (fp32)

Production config (build_nc_v3): 8 cores as a 4x2 grid — tokens split 4
ways (2048 rows/core), out-features split 2 ways (2048 cols/core), which
minimizes per-core HBM traffic (80 MiB: x 32 + W 32 + out 16).

Per core:
  - Host passes x.T / W.T shards (pure layout: transpose+slice; all math
    on device) so the contraction dim (IN) lands on SBUF partitions with
    natural DMAs; bias comes host-replicated to [128, O_LOC].
  - Binarize to +-0.5 in fp8e4 with a single DVE tensor_scalar op
    (is_ge(x,0) -> {0,1}, subtract 0.5); x cached binarized in SBUF
    (8 MiB), W streamed in 512-wide column tiles, double-buffered.
  - Matmuls run fp8 DoubleRow (K=256 per instr), emitted kb-block-major
    in mb=4 m-tile groups so the in-order PE stream tracks DMA arrival.
  - PSUM holds 0.25*S exactly (S = integer +-1 dot product); eviction is
    ACT Copy(scale=4) + DVE bias add -> bit-exact vs the fp32 reference.

Measured ~0.34-0.38 ms on 8 cores (PE floor ~0.235 ms, DMA floor
~0.235 ms; remainder is ramp + engine interaction).
"""

import os
import sys

sys.path.insert(0, "/opt/trn_rl_repo")

import numpy as np

import concourse.bass as bass
import concourse.mybir as mybir
import concourse.tile as tile
from concourse import bacc
from concourse.bass import ts

N_CORES = 8
P = 128

# Full problem shapes (hardcoded per contract)
N_TOK, D_IN, D_OUT = 8192, 4096, 4096


def build_nc(
    t_loc: int = N_TOK // N_CORES,
    d_in: int = D_IN,
    d_out: int = D_OUT,
    n_tile: int = 512,
    mm_dtype: mybir.dt = mybir.dt.bfloat16,
    double_row: bool = False,
    repeat: int = 1,
    n_pair: int = 1,
):
    """Build the per-core Bass program.

    Inputs (per core):
      xT   [d_in, t_loc] fp32   (x.T shard)
      wT   [d_in, d_out] fp32   (W.T, full)
      bias [128, d_out]  fp32   (host-replicated rows)
    Output:
      out  [t_loc, d_out] fp32
    """
    assert t_loc % P == 0 and d_in % P == 0 and d_out % n_tile == 0
    k_tiles = d_in // P
    m_tiles = t_loc // P
    n_tiles = d_out // n_tile
    if double_row:
        assert mm_dtype in (mybir.dt.float8e4, mybir.dt.float8e5)
        assert k_tiles % 2 == 0

    nc = bacc.Bacc("TRN2", target_bir_lowering=False, debug=False)

    xT = nc.dram_tensor("xT", [d_in, t_loc], mybir.dt.float32, kind="ExternalInput")
    wT = nc.dram_tensor("wT", [d_in, d_out], mybir.dt.float32, kind="ExternalInput")
    bias = nc.dram_tensor("bias", [P, d_out], mybir.dt.float32, kind="ExternalInput")
    out = nc.dram_tensor("out", [t_loc, d_out], mybir.dt.float32, kind="ExternalOutput")

    ge = mybir.AluOpType.is_ge
    sub = mybir.AluOpType.subtract
    add = mybir.AluOpType.add

    with tile.TileContext(nc) as tc:
        with (
            tc.tile_pool(name="const", bufs=1) as const_pool,
            tc.tile_pool(name="xbin_pool", bufs=1) as xbin_pool,
            tc.tile_pool(name="xstage", bufs=3) as xstage_pool,
            tc.tile_pool(name="wstage", bufs=6) as wstage_pool,
            tc.tile_pool(name="wbin", bufs=2 * n_pair) as wbin_pool,
            tc.tile_pool(name="evict", bufs=8) as evict_pool,
            tc.tile_pool(name="psum", bufs=6, space="PSUM") as psum_pool,
        ):
            # bias, replicated on host to [P, d_out]
            bias_sb = const_pool.tile([P, d_out], mybir.dt.float32, name="bias_sb")
            nc.sync.dma_start(bias_sb[:], bias[:])

            import contextlib

            rep_ctx = (
                tc.For_i(0, repeat, 1) if repeat > 1 else contextlib.nullcontext()
            )
            with rep_ctx:
                _body(
                    nc, tc, xT, wT, out, bias_sb,
                    xbin_pool, xstage_pool, wstage_pool, wbin_pool, evict_pool,
                    psum_pool, t_loc, n_tile, k_tiles, m_tiles, n_tiles,
                    mm_dtype, double_row, n_pair,
                )

    nc.compile()
    return nc


def _body(
    nc, tc, xT, wT, out, bias_sb,
    xbin_pool, xstage_pool, wstage_pool, wbin_pool, evict_pool,
    psum_pool, t_loc, n_tile, k_tiles, m_tiles, n_tiles,
    mm_dtype, double_row, n_pair=1,
):
    ge = mybir.AluOpType.is_ge
    sub = mybir.AluOpType.subtract
    add = mybir.AluOpType.add
    assert n_tiles % n_pair == 0

    # ---- load + binarize all of x.T into SBUF (mm_dtype, +-0.5) ----
    xbin = xbin_pool.tile([P, k_tiles, t_loc], mm_dtype, name="xbin")
    for k in range(k_tiles):
        xf = xstage_pool.tile([P, t_loc], mybir.dt.float32, name="xf")
        nc.sync.dma_start(xf[:], xT[ts(k, P), :])
        nc.vector.tensor_scalar(xbin[:, k, :], xf[:], 0.0, 0.5, ge, sub)

    # ---- stream W.T by group of n_pair n-tiles; matmul; evict ----
    for ng in range(n_tiles // n_pair):
        wbins = []
        for j in range(n_pair):
            n = ng * n_pair + j
            wbin = wbin_pool.tile([P, k_tiles, n_tile], mm_dtype, name="wbin")
            for k in range(k_tiles):
                wf = wstage_pool.tile([P, n_tile], mybir.dt.float32, name="wf")
                nc.sync.dma_start(wf[:], wT[ts(k, P), ts(n, n_tile)])
                nc.vector.tensor_scalar(wbin[:, k, :], wf[:], 0.0, 0.5, ge, sub)
            wbins.append(wbin)

        for m in range(m_tiles):
            psums = [
                psum_pool.tile([P, n_tile], mybir.dt.float32, name="psum")
                for _ in range(n_pair)
            ]
            if double_row:
                for k in range(0, k_tiles, 2):
                    for j in range(n_pair):
                        nc.tensor.matmul(
                            psums[j][:],
                            xbin[:, k : k + 2, ts(m, P)],
                            wbins[j][:, k : k + 2, :],
                            start=(k == 0),
                            stop=(k == k_tiles - 2),
                            perf_mode=mybir.MatmulPerfMode.DoubleRow,
                        )
            else:
                for k in range(k_tiles):
                    for j in range(n_pair):
                        nc.tensor.matmul(
                            psums[j][:],
                            xbin[:, k, ts(m, P)],
                            wbins[j][:, k, :],
                            start=(k == 0),
                            stop=(k == k_tiles - 1),
                        )
            for j in range(n_pair):
                n = ng * n_pair + j
                # out = 4 * psum + bias   (exact: psum = S/4, S integer)
                t1 = evict_pool.tile([P, n_tile], mybir.dt.float32, name="t1")
                nc.scalar.activation(
                    t1[:], psums[j][:], mybir.ActivationFunctionType.Copy, scale=4.0
                )
                ob = evict_pool.tile([P, n_tile], mybir.dt.float32, name="ob")
                nc.vector.tensor_tensor(
                    ob[:], t1[:], bias_sb[:, ts(n, n_tile)], add
                )
                nc.sync.dma_start(out[ts(m, P), ts(n, n_tile)], ob[:])


def build_nc_v3(
    t_loc: int = 2048,
    d_in: int = D_IN,
    d_out_loc: int = 2048,
    n_tile: int = 512,
    mm_dtype: mybir.dt = mybir.dt.float8e4,
    double_row: bool = True,
    repeat: int = 1,
    mb_size: int = 4,
    kb_tiles: int = 8,
    x_m_major: bool = False,
    w_sign_act: bool = False,
    xstage_bufs: int = 3,
    wstage_bufs: int = 6,
):
    """4x2-sharded variant: tokens split 4 ways, out-features 2 ways.

    Per core: xT [d_in, t_loc], wT [d_in, d_out_loc], bias [P, d_out_loc],
    out [t_loc, d_out_loc].  x binarized+cached in SBUF; W streamed by
    n-tile.  Matmuls are emitted kb-block-major so the in-order PE stream
    tracks DMA arrival order during the ramp.
    """
    assert t_loc % P == 0 and d_in % P == 0 and d_out_loc % n_tile == 0
    k_tiles = d_in // P
    m_tiles = t_loc // P
    n_tiles = d_out_loc // n_tile
    assert m_tiles % mb_size == 0 and k_tiles % kb_tiles == 0
    if double_row:
        assert kb_tiles % 2 == 0

    nc = bacc.Bacc("TRN2", target_bir_lowering=False, debug=False)
    xT = nc.dram_tensor("xT", [d_in, t_loc], mybir.dt.float32, kind="ExternalInput")
    wT = nc.dram_tensor("wT", [d_in, d_out_loc], mybir.dt.float32, kind="ExternalInput")
    bias = nc.dram_tensor("bias", [P, d_out_loc], mybir.dt.float32, kind="ExternalInput")
    out = nc.dram_tensor(
        "out", [t_loc, d_out_loc], mybir.dt.float32, kind="ExternalOutput"
    )

    ge = mybir.AluOpType.is_ge
    sub = mybir.AluOpType.subtract
    add = mybir.AluOpType.add
    k_step = 2 if double_row else 1
    perf = mybir.MatmulPerfMode.DoubleRow if double_row else None
    kb_blocks = k_tiles // kb_tiles

    with tile.TileContext(nc) as tc:
        with (
            tc.tile_pool(name="const", bufs=1) as const_pool,
            tc.tile_pool(name="xbin_pool", bufs=1) as xbin_pool,
            tc.tile_pool(name="xstage", bufs=xstage_bufs) as xstage_pool,
            tc.tile_pool(name="wstage", bufs=wstage_bufs) as wstage_pool,
            tc.tile_pool(name="wbin", bufs=2) as wbin_pool,
            tc.tile_pool(name="evict", bufs=8) as evict_pool,
            tc.tile_pool(name="psum", bufs=8, space="PSUM") as psum_pool,
        ):
            bias_sb = const_pool.tile([P, d_out_loc], mybir.dt.float32, name="bias_sb")
            nc.sync.dma_start(bias_sb[:], bias[:])

            import contextlib

            rep_ctx = (
                tc.For_i(0, repeat, 1) if repeat > 1 else contextlib.nullcontext()
            )
            with rep_ctx:
                xbin = xbin_pool.tile([P, k_tiles, t_loc], mm_dtype, name="xbin")
                if x_m_major:
                    # Load x by token-block (all k per block) so the first
                    # matmul group's operands arrive early — matches the
                    # (mb, kb) consumption order of the in-order PE stream.
                    mblk = mb_size * P
                    for mb0 in range(t_loc // mblk):
                        for k in range(k_tiles):
                            xf = xstage_pool.tile(
                                [P, mblk], mybir.dt.float32, name="xf"
                            )
                            nc.sync.dma_start(
                                xf[:], xT[ts(k, P), ts(mb0, mblk)]
                            )
                            nc.vector.tensor_scalar(
                                xbin[:, k, ts(mb0, mblk)], xf[:], 0.0, 0.5, ge, sub
                            )
                else:
                    for k in range(k_tiles):
                        xf = xstage_pool.tile(
                            [P, t_loc], mybir.dt.float32, name="xf"
                        )
                        nc.sync.dma_start(xf[:], xT[ts(k, P), :])
                        nc.vector.tensor_scalar(
                            xbin[:, k, :], xf[:], 0.0, 0.5, ge, sub
                        )

                for n in range(n_tiles):
                    wbin = wbin_pool.tile([P, k_tiles, n_tile], mm_dtype, name="wbin")
                    for k in range(k_tiles):
                        wf = wstage_pool.tile(
                            [P, n_tile], mybir.dt.float32, name="wf"
                        )
                        nc.sync.dma_start(wf[:], wT[ts(k, P), ts(n, n_tile)])
                        if w_sign_act:
                            nc.scalar.sign(wbin[:, k, :], wf[:])
                        else:
                            nc.vector.tensor_scalar(
                                wbin[:, k, :], wf[:], 0.0, 0.5, ge, sub
                            )

                    for mb in range(m_tiles // mb_size):
                        psums = [
                            psum_pool.tile(
                                [P, n_tile], mybir.dt.float32, name="psum"
                            )
                            for _ in range(mb_size)
                        ]
                        for kb in range(kb_blocks):
                            for mi in range(mb_size):
                                m = mb * mb_size + mi
                                for kp in range(0, kb_tiles, k_step):
                                    k = kb * kb_tiles + kp
                                    nc.tensor.matmul(
                                        psums[mi][:],
                                        xbin[:, k : k + k_step, ts(m, P)],
                                        wbin[:, k : k + k_step, :],
                                        start=(kb == 0 and kp == 0),
                                        stop=(
                                            kb == kb_blocks - 1
                                            and kp == kb_tiles - k_step
                                        ),
                                        perf_mode=perf,
                                    )
                        for mi in range(mb_size):
                            m = mb * mb_size + mi
                            t1 = evict_pool.tile(
                                [P, n_tile], mybir.dt.float32, name="t1"
                            )
                            if w_sign_act:
                                # ACT is busy with W Sign; scale on DVE
                                nc.vector.tensor_scalar_mul(
                                    t1[:], psums[mi][:], 2.0
                                )
                            else:
                                nc.scalar.activation(
                                    t1[:],
                                    psums[mi][:],
                                    mybir.ActivationFunctionType.Copy,
                                    scale=4.0,
                                )
                            ob = evict_pool.tile(
                                [P, n_tile], mybir.dt.float32, name="ob"
                            )
                            nc.vector.tensor_tensor(
                                ob[:], t1[:], bias_sb[:, ts(n, n_tile)], add
                            )
                            nc.sync.dma_start(out[ts(m, P), ts(n, n_tile)], ob[:])

    nc.compile()
    return nc


def build_nc_v4(
    t_loc: int = 2048,
    d_in: int = D_IN,
    d_out_loc: int = 2048,
    o_group: int = 512,
    t_tile: int = 512,
    mm_dtype: mybir.dt = mybir.dt.float8e4,
    double_row: bool = True,
    repeat: int = 1,
    kb_tiles: int = 8,
    diag: str = "full",  # "full" | "no_w" | "no_x" | "no_bin" (timing diagnostics)
):
    """out.T orientation: W chunks are the stationary operand, x moving.

    Each LDWEIGHTS (wbin [128, 2, 128]) serves t_loc/t_tile consecutive
    matmuls (moving over token tiles), hiding the DoubleRow weight-load
    cost.  PSUM is [outf, tokens]; eviction is a single DVE
    tensor_scalar(psum*4 + bias[p]) with per-partition bias AP; output is
    written as out.T [d_out_loc, t_loc] and un-transposed on host.

    Inputs per core: xT [d_in, t_loc], wT [d_in, d_out_loc],
    biasT [P, d_out_loc//P] (bias.reshape(-1, P).T), outT [d_out_loc, t_loc].
    """
    assert t_loc % t_tile == 0 and d_in % P == 0 and d_out_loc % o_group == 0
    k_tiles = d_in // P
    o_blocks = d_out_loc // P
    o_per_g = o_group // P
    t_blocks = t_loc // t_tile
    kb_blocks = k_tiles // kb_tiles
    if double_row:
        assert kb_tiles % 2 == 0
    k_step = 2 if double_row else 1
    perf = mybir.MatmulPerfMode.DoubleRow if double_row else None

    nc = bacc.Bacc("TRN2", target_bir_lowering=False, debug=False)
    xT = nc.dram_tensor("xT", [d_in, t_loc], mybir.dt.float32, kind="ExternalInput")
    wT = nc.dram_tensor("wT", [d_in, d_out_loc], mybir.dt.float32, kind="ExternalInput")
    biasT = nc.dram_tensor(
        "biasT", [P, o_blocks], mybir.dt.float32, kind="ExternalInput"
    )
    outT = nc.dram_tensor(
        "outT", [d_out_loc, t_loc], mybir.dt.float32, kind="ExternalOutput"
    )

    ge = mybir.AluOpType.is_ge
    sub = mybir.AluOpType.subtract
    mult = mybir.AluOpType.mult
    add = mybir.AluOpType.add

    with tile.TileContext(nc) as tc:
        with (
            tc.tile_pool(name="const", bufs=1) as const_pool,
            tc.tile_pool(name="xbin_pool", bufs=1) as xbin_pool,
            tc.tile_pool(name="xstage", bufs=3) as xstage_pool,
            tc.tile_pool(name="wstage", bufs=6) as wstage_pool,
            tc.tile_pool(name="wbin", bufs=2) as wbin_pool,
            tc.tile_pool(name="evict", bufs=8) as evict_pool,
            tc.tile_pool(name="psum", bufs=8, space="PSUM") as psum_pool,
        ):
            bias_sb = const_pool.tile([P, o_blocks], mybir.dt.float32, name="bias_sb")
            nc.sync.dma_start(bias_sb[:], biasT[:])

            # Diagnostic-only: pre-filled operand tiles living outside the
            # timed repeat loop.
            xbin_fixed = wbin_fixed = None
            if diag in ("no_x", "no_bin", "pe_only"):
                xbin_fixed = const_pool.tile(
                    [P, k_tiles, t_loc], mm_dtype, name="xbin_fixed"
                )
                for _k in range(k_tiles):
                    nc.any.memset(xbin_fixed[:, _k, :], 0.5)
            if diag in ("no_w", "no_bin", "pe_only"):
                wbin_fixed = const_pool.tile(
                    [P, k_tiles, o_group], mm_dtype, name="wbin_fixed"
                )
                nc.any.memset(wbin_fixed[:], 0.5)

            # keep skipped inputs referenced so walrus accepts the NEFF
            if diag in ("no_x", "pe_only"):
                dummy_x = const_pool.tile([P, 16], mybir.dt.float32, name="dummy_x")
                nc.sync.dma_start(dummy_x[:], xT[:P, :16])
            if diag in ("no_w", "pe_only"):
                dummy_w = const_pool.tile([P, 16], mybir.dt.float32, name="dummy_w")
                nc.sync.dma_start(dummy_w[:], wT[:P, :16])

            import contextlib

            rep_ctx = (
                tc.For_i(0, repeat, 1) if repeat > 1 else contextlib.nullcontext()
            )
            with rep_ctx:
                if xbin_fixed is not None:
                    xbin = xbin_fixed
                else:
                    xbin = xbin_pool.tile([P, k_tiles, t_loc], mm_dtype, name="xbin")
                if diag not in ("no_x", "pe_only"):
                    for k in range(k_tiles):
                        xf = xstage_pool.tile(
                            [P, t_loc], mybir.dt.float32, name="xf"
                        )
                        nc.sync.dma_start(xf[:], xT[ts(k, P), :])
                        if diag != "no_bin":
                            nc.vector.tensor_scalar(
                                xbin[:, k, :], xf[:], 0.0, 0.5, ge, sub
                            )

                for og in range(d_out_loc // o_group):
                    if wbin_fixed is not None:
                        wbin = wbin_fixed
                    else:
                        wbin = wbin_pool.tile(
                            [P, k_tiles, o_group], mm_dtype, name="wbin"
                        )
                    if diag not in ("no_w", "pe_only"):
                        for k in range(k_tiles):
                            wf = wstage_pool.tile(
                                [P, o_group], mybir.dt.float32, name="wf"
                            )
                            nc.sync.dma_start(wf[:], wT[ts(k, P), ts(og, o_group)])
                            if diag != "no_bin":
                                nc.vector.tensor_scalar(
                                    wbin[:, k, :], wf[:], 0.0, 0.5, ge, sub
                                )

                    for oi in range(o_per_g):
                        o = og * o_per_g + oi
                        psums = [
                            psum_pool.tile([P, t_tile], mybir.dt.float32, name="psum")
                            for _ in range(t_blocks)
                        ]
                        for kb in range(kb_blocks):
                            for kp in range(0, kb_tiles, k_step):
                                k = kb * kb_tiles + kp
                                for t in range(t_blocks):
                                    nc.tensor.matmul(
                                        psums[t][:],
                                        wbin[:, k : k + k_step, ts(oi, P)],
                                        xbin[:, k : k + k_step, ts(t, t_tile)],
                                        start=(kb == 0 and kp == 0),
                                        stop=(
                                            kb == kb_blocks - 1
                                            and kp == kb_tiles - k_step
                                        ),
                                        perf_mode=perf,
                                    )
                        for t in range(t_blocks):
                            ob = evict_pool.tile(
                                [P, t_tile], mybir.dt.float32, name="ob"
                            )
                            # out = psum*4 + bias[p]  (exact; bias per-partition)
                            nc.vector.tensor_scalar(
                                ob[:],
                                psums[t][:],
                                4.0,
                                bias_sb[:, o : o + 1],
                                mult,
                                add,
                            )
                            nc.sync.dma_start(outT[ts(o, P), ts(t, t_tile)], ob[:])

    nc.compile()
    return nc


_NC_CACHE: dict = {}

# production sharding: 4-way tokens x 2-way out-features
T_GRP, O_GRP = 4, 2
T_LOC = N_TOK // T_GRP  # 2048
O_LOC = D_OUT // O_GRP  # 2048


def _get_nc(key=("v3",)):
    if key not in _NC_CACHE:
        _NC_CACHE[key] = build_nc_v3(
            t_loc=T_LOC, d_out_loc=O_LOC, xstage_bufs=5, wstage_bufs=12
        )
    return _NC_CACHE[key]


def make_in_maps(x, fp_weight, fp_bias):
    """Host-side sharding (layout only: transpose + slice + replicate)."""
    xT = np.ascontiguousarray(np.asarray(x, dtype=np.float32).T)  # [D_IN, N_TOK]
    wT = np.ascontiguousarray(np.asarray(fp_weight, dtype=np.float32).T)
    bias = np.asarray(fp_bias, dtype=np.float32)
    in_maps = []
    for c in range(N_CORES):
        i, j = divmod(c, O_GRP)
        in_maps.append(
            {
                "xT": np.ascontiguousarray(xT[:, i * T_LOC : (i + 1) * T_LOC]),
                "wT": np.ascontiguousarray(wT[:, j * O_LOC : (j + 1) * O_LOC]),
                "bias": np.ascontiguousarray(
                    np.broadcast_to(
                        bias[None, j * O_LOC : (j + 1) * O_LOC], (P, O_LOC)
                    )
                ),
            }
        )
    return in_maps


def assemble(results) -> np.ndarray:
    out = np.empty((N_TOK, D_OUT), np.float32)
    for c in range(N_CORES):
        i, j = divmod(c, O_GRP)
        out[i * T_LOC : (i + 1) * T_LOC, j * O_LOC : (j + 1) * O_LOC] = results[c][
            "out"
        ]
    return out


def kernel(x: np.ndarray, fp_weight: np.ndarray, fp_bias: np.ndarray) -> np.ndarray:
    assert x.shape == (N_TOK, D_IN) and fp_weight.shape == (D_OUT, D_IN)
    from concourse.bass_utils import run_bass_kernel_spmd

    nc = _get_nc()
    in_maps = make_in_maps(x, fp_weight, fp_bias)
    res = run_bass_kernel_spmd(nc, in_maps, core_ids=list(range(N_CORES)))
    return assemble(res.results)



# revision 29
# speedup vs baseline: 1.1279x; 1.1279x over previous
"""BinarizedLinear Trainium2 kernel.

out = sign(x) @ sign(W).T + bias,  x:[8192,4096] W:[4096,4096] bias:[4096] (fp32)

Production config (build_nc_v6): 8 cores as a 4x2 grid — tokens split 4
ways (2048 rows/core), out-features split 2 ways (2048 cols/core), which
minimizes per-core HBM traffic (72 MiB: x 32 + W 32 + out-fp16 8).

Per core (v6):
  - Host relayouts x.T / W.T shards (pure layout: transpose/reshape/
    slice; all math on device) into [blk, kg, 128, chunk] fp32 so every
    input DMA is a fully contiguous 0.5-1 MiB block — measured read
    stream hits ~361 GB/s (HBM-per-NC cap ~358).
  - Rectangle-growth delivery/emission: load W-col0 + x-blk0 chunk-
    interleaved, then alternate new W columns and x blocks; matmul
    groups (mb, n) are emitted exactly as the growing W-cols x x-blks
    rectangle enables them, so the in-order PE stream tracks DMA
    arrival and the serial ramp is minimized.  All 4 W columns stay
    resident (fp8, 4 MiB); x is cached per-mb-block in 4 tiles.
  - Binarize to +-0.5 in fp8e4 on DVE (is_ge -> {0,1} - 0.5; exact,
    including sign(0)=+1); 3-4-deep stage pools keep the read pipeline
    full (2-deep stalls it on the ~2us DMA-completion latency).
  - Matmuls run fp8 DoubleRow (K=256/instr, N=512, ~227 ns/MM measured;
    PE+evict floor ~232 us for the 1024-MM stream).
  - PSUM holds S/4 exactly (S = integer +-1 dot product); eviction is
    ACT Copy(scale=4) + DVE bias add, written as fp16 on the ACT HWDGE
    ring (host upcasts; |S| << 2048 so fp16 keeps S exact and only
    rounds the small bias: rel err ~1e-4).  Writes measured ~200 GB/s,
    so halving write bytes matters; loads stay on the SP ring to avoid
    head-of-line blocking.

Component floors (R-slope, per iteration): reads+binarize 186 us,
PE+evict+writes 232 us.  Production measured ~305 us (was 363 us
baseline v3); remainder is the DMA-paced ramp (first 16 MiB before the
first group can retire) plus read/write turnaround.
"""

import os
import sys

sys.path.insert(0, "/opt/trn_rl_repo")

import numpy as np

import concourse.bass as bass
import concourse.mybir as mybir
import concourse.tile as tile
from concourse import bacc
from concourse.bass import ts

N_CORES = 8
P = 128

# Full problem shapes (hardcoded per contract)
N_TOK, D_IN, D_OUT = 8192, 4096, 4096


def build_nc(
    t_loc: int = N_TOK // N_CORES,
    d_in: int = D_IN,
    d_out: int = D_OUT,
    n_tile: int = 512,
    mm_dtype: mybir.dt = mybir.dt.bfloat16,
    double_row: bool = False,
    repeat: int = 1,
    n_pair: int = 1,
):
    """Build the per-core Bass program.

    Inputs (per core):
      xT   [d_in, t_loc] fp32   (x.T shard)
      wT   [d_in, d_out] fp32   (W.T, full)
      bias [128, d_out]  fp32   (host-replicated rows)
    Output:
      out  [t_loc, d_out] fp32
    """
    assert t_loc % P == 0 and d_in % P == 0 and d_out % n_tile == 0
    k_tiles = d_in // P
    m_tiles = t_loc // P
    n_tiles = d_out // n_tile
    if double_row:
        assert mm_dtype in (mybir.dt.float8e4, mybir.dt.float8e5)
        assert k_tiles % 2 == 0

    nc = bacc.Bacc("TRN2", target_bir_lowering=False, debug=False)

    xT = nc.dram_tensor("xT", [d_in, t_loc], mybir.dt.float32, kind="ExternalInput")
    wT = nc.dram_tensor("wT", [d_in, d_out], mybir.dt.float32, kind="ExternalInput")
    bias = nc.dram_tensor("bias", [P, d_out], mybir.dt.float32, kind="ExternalInput")
    out = nc.dram_tensor("out", [t_loc, d_out], mybir.dt.float32, kind="ExternalOutput")

    ge = mybir.AluOpType.is_ge
    sub = mybir.AluOpType.subtract
    add = mybir.AluOpType.add

    with tile.TileContext(nc) as tc:
        with (
            tc.tile_pool(name="const", bufs=1) as const_pool,
            tc.tile_pool(name="xbin_pool", bufs=1) as xbin_pool,
            tc.tile_pool(name="xstage", bufs=3) as xstage_pool,
            tc.tile_pool(name="wstage", bufs=6) as wstage_pool,
            tc.tile_pool(name="wbin", bufs=2 * n_pair) as wbin_pool,
            tc.tile_pool(name="evict", bufs=8) as evict_pool,
            tc.tile_pool(name="psum", bufs=6, space="PSUM") as psum_pool,
        ):
            # bias, replicated on host to [P, d_out]
            bias_sb = const_pool.tile([P, d_out], mybir.dt.float32, name="bias_sb")
            nc.sync.dma_start(bias_sb[:], bias[:])

            import contextlib

            rep_ctx = (
                tc.For_i(0, repeat, 1) if repeat > 1 else contextlib.nullcontext()
            )
            with rep_ctx:
                _body(
                    nc, tc, xT, wT, out, bias_sb,
                    xbin_pool, xstage_pool, wstage_pool, wbin_pool, evict_pool,
                    psum_pool, t_loc, n_tile, k_tiles, m_tiles, n_tiles,
                    mm_dtype, double_row, n_pair,
                )

    nc.compile()
    return nc


def _body(
    nc, tc, xT, wT, out, bias_sb,
    xbin_pool, xstage_pool, wstage_pool, wbin_pool, evict_pool,
    psum_pool, t_loc, n_tile, k_tiles, m_tiles, n_tiles,
    mm_dtype, double_row, n_pair=1,
):
    ge = mybir.AluOpType.is_ge
    sub = mybir.AluOpType.subtract
    add = mybir.AluOpType.add
    assert n_tiles % n_pair == 0

    # ---- load + binarize all of x.T into SBUF (mm_dtype, +-0.5) ----
    xbin = xbin_pool.tile([P, k_tiles, t_loc], mm_dtype, name="xbin")
    for k in range(k_tiles):
        xf = xstage_pool.tile([P, t_loc], mybir.dt.float32, name="xf")
        nc.sync.dma_start(xf[:], xT[ts(k, P), :])
        nc.vector.tensor_scalar(xbin[:, k, :], xf[:], 0.0, 0.5, ge, sub)

    # ---- stream W.T by group of n_pair n-tiles; matmul; evict ----
    for ng in range(n_tiles // n_pair):
        wbins = []
        for j in range(n_pair):
            n = ng * n_pair + j
            wbin = wbin_pool.tile([P, k_tiles, n_tile], mm_dtype, name="wbin")
            for k in range(k_tiles):
                wf = wstage_pool.tile([P, n_tile], mybir.dt.float32, name="wf")
                nc.sync.dma_start(wf[:], wT[ts(k, P), ts(n, n_tile)])
                nc.vector.tensor_scalar(wbin[:, k, :], wf[:], 0.0, 0.5, ge, sub)
            wbins.append(wbin)

        for m in range(m_tiles):
            psums = [
                psum_pool.tile([P, n_tile], mybir.dt.float32, name="psum")
                for _ in range(n_pair)
            ]
            if double_row:
                for k in range(0, k_tiles, 2):
                    for j in range(n_pair):
                        nc.tensor.matmul(
                            psums[j][:],
                            xbin[:, k : k + 2, ts(m, P)],
                            wbins[j][:, k : k + 2, :],
                            start=(k == 0),
                            stop=(k == k_tiles - 2),
                            perf_mode=mybir.MatmulPerfMode.DoubleRow,
                        )
            else:
                for k in range(k_tiles):
                    for j in range(n_pair):
                        nc.tensor.matmul(
                            psums[j][:],
                            xbin[:, k, ts(m, P)],
                            wbins[j][:, k, :],
                            start=(k == 0),
                            stop=(k == k_tiles - 1),
                        )
            for j in range(n_pair):
                n = ng * n_pair + j
                # out = 4 * psum + bias   (exact: psum = S/4, S integer)
                t1 = evict_pool.tile([P, n_tile], mybir.dt.float32, name="t1")
                nc.scalar.activation(
                    t1[:], psums[j][:], mybir.ActivationFunctionType.Copy, scale=4.0
                )
                ob = evict_pool.tile([P, n_tile], mybir.dt.float32, name="ob")
                nc.vector.tensor_tensor(
                    ob[:], t1[:], bias_sb[:, ts(n, n_tile)], add
                )
                nc.sync.dma_start(out[ts(m, P), ts(n, n_tile)], ob[:])


def build_nc_v3(
    t_loc: int = 2048,
    d_in: int = D_IN,
    d_out_loc: int = 2048,
    n_tile: int = 512,
    mm_dtype: mybir.dt = mybir.dt.float8e4,
    double_row: bool = True,
    repeat: int = 1,
    mb_size: int = 4,
    kb_tiles: int = 8,
    x_m_major: bool = False,
    w_sign_act: bool = False,
    xstage_bufs: int = 3,
    wstage_bufs: int = 6,
    diag: str = "full",  # "full" | "dma_only" | "no_in" (timing diagnostics)
):
    """4x2-sharded variant: tokens split 4 ways, out-features 2 ways.

    Per core: xT [d_in, t_loc], wT [d_in, d_out_loc], bias [P, d_out_loc],
    out [t_loc, d_out_loc].  x binarized+cached in SBUF; W streamed by
    n-tile.  Matmuls are emitted kb-block-major so the in-order PE stream
    tracks DMA arrival order during the ramp.
    """
    assert t_loc % P == 0 and d_in % P == 0 and d_out_loc % n_tile == 0
    k_tiles = d_in // P
    m_tiles = t_loc // P
    n_tiles = d_out_loc // n_tile
    assert m_tiles % mb_size == 0 and k_tiles % kb_tiles == 0
    if double_row:
        assert kb_tiles % 2 == 0

    nc = bacc.Bacc("TRN2", target_bir_lowering=False, debug=False)
    xT = nc.dram_tensor("xT", [d_in, t_loc], mybir.dt.float32, kind="ExternalInput")
    wT = nc.dram_tensor("wT", [d_in, d_out_loc], mybir.dt.float32, kind="ExternalInput")
    bias = nc.dram_tensor("bias", [P, d_out_loc], mybir.dt.float32, kind="ExternalInput")
    out = nc.dram_tensor(
        "out", [t_loc, d_out_loc], mybir.dt.float32, kind="ExternalOutput"
    )

    ge = mybir.AluOpType.is_ge
    sub = mybir.AluOpType.subtract
    add = mybir.AluOpType.add
    k_step = 2 if double_row else 1
    perf = mybir.MatmulPerfMode.DoubleRow if double_row else None
    kb_blocks = k_tiles // kb_tiles

    with tile.TileContext(nc) as tc:
        with (
            tc.tile_pool(name="const", bufs=1) as const_pool,
            tc.tile_pool(name="xbin_pool", bufs=1) as xbin_pool,
            tc.tile_pool(name="xstage", bufs=xstage_bufs) as xstage_pool,
            tc.tile_pool(name="wstage", bufs=wstage_bufs) as wstage_pool,
            tc.tile_pool(name="wbin", bufs=2) as wbin_pool,
            tc.tile_pool(name="evict", bufs=8) as evict_pool,
            tc.tile_pool(name="psum", bufs=8, space="PSUM") as psum_pool,
        ):
            bias_sb = const_pool.tile([P, d_out_loc], mybir.dt.float32, name="bias_sb")
            nc.sync.dma_start(bias_sb[:], bias[:])

            # Diagnostic-only fixed tiles (outside the timed repeat loop).
            xbin_fixed = wbin_fixed = ob_fixed = None
            if diag == "no_in":
                xbin_fixed = const_pool.tile(
                    [P, k_tiles, t_loc], mm_dtype, name="xbin_fixed"
                )
                for _k in range(k_tiles):
                    nc.any.memset(xbin_fixed[:, _k, :], 0.5)
                wbin_fixed = const_pool.tile(
                    [P, k_tiles, n_tile], mm_dtype, name="wbin_fixed"
                )
                nc.any.memset(wbin_fixed[:], 0.5)
                dummy_x = const_pool.tile([P, 16], mybir.dt.float32, name="dummy_x")
                nc.sync.dma_start(dummy_x[:], xT[:P, :16])
                dummy_w = const_pool.tile([P, 16], mybir.dt.float32, name="dummy_w")
                nc.sync.dma_start(dummy_w[:], wT[:P, :16])
            if diag == "dma_only":
                ob_fixed = const_pool.tile([P, n_tile], mybir.dt.float32, name="ob_fixed")
                nc.any.memset(ob_fixed[:], 1.0)

            import contextlib

            rep_ctx = (
                tc.For_i(0, repeat, 1) if repeat > 1 else contextlib.nullcontext()
            )
            with rep_ctx:
                xbin = (
                    xbin_fixed
                    if diag == "no_in"
                    else xbin_pool.tile([P, k_tiles, t_loc], mm_dtype, name="xbin")
                )
                if diag == "no_in":
                    pass
                elif x_m_major:
                    # Load x by token-block (all k per block) so the first
                    # matmul group's operands arrive early — matches the
                    # (mb, kb) consumption order of the in-order PE stream.
                    mblk = mb_size * P
                    for mb0 in range(t_loc // mblk):
                        for k in range(k_tiles):
                            xf = xstage_pool.tile(
                                [P, mblk], mybir.dt.float32, name="xf"
                            )
                            nc.sync.dma_start(
                                xf[:], xT[ts(k, P), ts(mb0, mblk)]
                            )
                            nc.vector.tensor_scalar(
                                xbin[:, k, ts(mb0, mblk)], xf[:], 0.0, 0.5, ge, sub
                            )
                else:
                    for k in range(k_tiles):
                        xf = xstage_pool.tile(
                            [P, t_loc], mybir.dt.float32, name="xf"
                        )
                        nc.sync.dma_start(xf[:], xT[ts(k, P), :])
                        nc.vector.tensor_scalar(
                            xbin[:, k, :], xf[:], 0.0, 0.5, ge, sub
                        )

                for n in range(n_tiles):
                    wbin = (
                        wbin_fixed
                        if diag == "no_in"
                        else wbin_pool.tile([P, k_tiles, n_tile], mm_dtype, name="wbin")
                    )
                    if diag != "no_in":
                        for k in range(k_tiles):
                            wf = wstage_pool.tile(
                                [P, n_tile], mybir.dt.float32, name="wf"
                            )
                            nc.sync.dma_start(wf[:], wT[ts(k, P), ts(n, n_tile)])
                            if w_sign_act:
                                nc.scalar.sign(wbin[:, k, :], wf[:])
                            else:
                                nc.vector.tensor_scalar(
                                    wbin[:, k, :], wf[:], 0.0, 0.5, ge, sub
                                )

                    if diag == "dma_only":
                        for mb in range(m_tiles // mb_size):
                            for mi in range(mb_size):
                                m = mb * mb_size + mi
                                nc.sync.dma_start(
                                    out[ts(m, P), ts(n, n_tile)], ob_fixed[:]
                                )
                        continue

                    for mb in range(m_tiles // mb_size):
                        psums = [
                            psum_pool.tile(
                                [P, n_tile], mybir.dt.float32, name="psum"
                            )
                            for _ in range(mb_size)
                        ]
                        for kb in range(kb_blocks):
                            for mi in range(mb_size):
                                m = mb * mb_size + mi
                                for kp in range(0, kb_tiles, k_step):
                                    k = kb * kb_tiles + kp
                                    nc.tensor.matmul(
                                        psums[mi][:],
                                        xbin[:, k : k + k_step, ts(m, P)],
                                        wbin[:, k : k + k_step, :],
                                        start=(kb == 0 and kp == 0),
                                        stop=(
                                            kb == kb_blocks - 1
                                            and kp == kb_tiles - k_step
                                        ),
                                        perf_mode=perf,
                                    )
                        for mi in range(mb_size):
                            m = mb * mb_size + mi
                            t1 = evict_pool.tile(
                                [P, n_tile], mybir.dt.float32, name="t1"
                            )
                            if w_sign_act:
                                # ACT is busy with W Sign; scale on DVE
                                nc.vector.tensor_scalar_mul(
                                    t1[:], psums[mi][:], 2.0
                                )
                            else:
                                nc.scalar.activation(
                                    t1[:],
                                    psums[mi][:],
                                    mybir.ActivationFunctionType.Copy,
                                    scale=4.0,
                                )
                            ob = evict_pool.tile(
                                [P, n_tile], mybir.dt.float32, name="ob"
                            )
                            nc.vector.tensor_tensor(
                                ob[:], t1[:], bias_sb[:, ts(n, n_tile)], add
                            )
                            nc.sync.dma_start(out[ts(m, P), ts(n, n_tile)], ob[:])

    nc.compile()
    return nc


def build_nc_v4(
    t_loc: int = 2048,
    d_in: int = D_IN,
    d_out_loc: int = 2048,
    o_group: int = 512,
    t_tile: int = 512,
    mm_dtype: mybir.dt = mybir.dt.float8e4,
    double_row: bool = True,
    repeat: int = 1,
    kb_tiles: int = 8,
    diag: str = "full",  # "full" | "no_w" | "no_x" | "no_bin" (timing diagnostics)
):
    """out.T orientation: W chunks are the stationary operand, x moving.

    Each LDWEIGHTS (wbin [128, 2, 128]) serves t_loc/t_tile consecutive
    matmuls (moving over token tiles), hiding the DoubleRow weight-load
    cost.  PSUM is [outf, tokens]; eviction is a single DVE
    tensor_scalar(psum*4 + bias[p]) with per-partition bias AP; output is
    written as out.T [d_out_loc, t_loc] and un-transposed on host.

    Inputs per core: xT [d_in, t_loc], wT [d_in, d_out_loc],
    biasT [P, d_out_loc//P] (bias.reshape(-1, P).T), outT [d_out_loc, t_loc].
    """
    assert t_loc % t_tile == 0 and d_in % P == 0 and d_out_loc % o_group == 0
    k_tiles = d_in // P
    o_blocks = d_out_loc // P
    o_per_g = o_group // P
    t_blocks = t_loc // t_tile
    kb_blocks = k_tiles // kb_tiles
    if double_row:
        assert kb_tiles % 2 == 0
    k_step = 2 if double_row else 1
    perf = mybir.MatmulPerfMode.DoubleRow if double_row else None

    nc = bacc.Bacc("TRN2", target_bir_lowering=False, debug=False)
    xT = nc.dram_tensor("xT", [d_in, t_loc], mybir.dt.float32, kind="ExternalInput")
    wT = nc.dram_tensor("wT", [d_in, d_out_loc], mybir.dt.float32, kind="ExternalInput")
    biasT = nc.dram_tensor(
        "biasT", [P, o_blocks], mybir.dt.float32, kind="ExternalInput"
    )
    outT = nc.dram_tensor(
        "outT", [d_out_loc, t_loc], mybir.dt.float32, kind="ExternalOutput"
    )

    ge = mybir.AluOpType.is_ge
    sub = mybir.AluOpType.subtract
    mult = mybir.AluOpType.mult
    add = mybir.AluOpType.add

    with tile.TileContext(nc) as tc:
        with (
            tc.tile_pool(name="const", bufs=1) as const_pool,
            tc.tile_pool(name="xbin_pool", bufs=1) as xbin_pool,
            tc.tile_pool(name="xstage", bufs=3) as xstage_pool,
            tc.tile_pool(name="wstage", bufs=6) as wstage_pool,
            tc.tile_pool(name="wbin", bufs=2) as wbin_pool,
            tc.tile_pool(name="evict", bufs=8) as evict_pool,
            tc.tile_pool(name="psum", bufs=8, space="PSUM") as psum_pool,
        ):
            bias_sb = const_pool.tile([P, o_blocks], mybir.dt.float32, name="bias_sb")
            nc.sync.dma_start(bias_sb[:], biasT[:])

            # Diagnostic-only: pre-filled operand tiles living outside the
            # timed repeat loop.
            xbin_fixed = wbin_fixed = None
            if diag in ("no_x", "no_bin", "pe_only"):
                xbin_fixed = const_pool.tile(
                    [P, k_tiles, t_loc], mm_dtype, name="xbin_fixed"
                )
                for _k in range(k_tiles):
                    nc.any.memset(xbin_fixed[:, _k, :], 0.5)
            if diag in ("no_w", "no_bin", "pe_only"):
                wbin_fixed = const_pool.tile(
                    [P, k_tiles, o_group], mm_dtype, name="wbin_fixed"
                )
                nc.any.memset(wbin_fixed[:], 0.5)

            # keep skipped inputs referenced so walrus accepts the NEFF
            if diag in ("no_x", "pe_only"):
                dummy_x = const_pool.tile([P, 16], mybir.dt.float32, name="dummy_x")
                nc.sync.dma_start(dummy_x[:], xT[:P, :16])
            if diag in ("no_w", "pe_only"):
                dummy_w = const_pool.tile([P, 16], mybir.dt.float32, name="dummy_w")
                nc.sync.dma_start(dummy_w[:], wT[:P, :16])

            import contextlib

            rep_ctx = (
                tc.For_i(0, repeat, 1) if repeat > 1 else contextlib.nullcontext()
            )
            with rep_ctx:
                if xbin_fixed is not None:
                    xbin = xbin_fixed
                else:
                    xbin = xbin_pool.tile([P, k_tiles, t_loc], mm_dtype, name="xbin")
                if diag not in ("no_x", "pe_only"):
                    for k in range(k_tiles):
                        xf = xstage_pool.tile(
                            [P, t_loc], mybir.dt.float32, name="xf"
                        )
                        nc.sync.dma_start(xf[:], xT[ts(k, P), :])
                        if diag != "no_bin":
                            nc.vector.tensor_scalar(
                                xbin[:, k, :], xf[:], 0.0, 0.5, ge, sub
                            )

                for og in range(d_out_loc // o_group):
                    if wbin_fixed is not None:
                        wbin = wbin_fixed
                    else:
                        wbin = wbin_pool.tile(
                            [P, k_tiles, o_group], mm_dtype, name="wbin"
                        )
                    if diag not in ("no_w", "pe_only"):
                        for k in range(k_tiles):
                            wf = wstage_pool.tile(
                                [P, o_group], mybir.dt.float32, name="wf"
                            )
                            nc.sync.dma_start(wf[:], wT[ts(k, P), ts(og, o_group)])
                            if diag != "no_bin":
                                nc.vector.tensor_scalar(
                                    wbin[:, k, :], wf[:], 0.0, 0.5, ge, sub
                                )

                    for oi in range(o_per_g):
                        o = og * o_per_g + oi
                        psums = [
                            psum_pool.tile([P, t_tile], mybir.dt.float32, name="psum")
                            for _ in range(t_blocks)
                        ]
                        for kb in range(kb_blocks):
                            for kp in range(0, kb_tiles, k_step):
                                k = kb * kb_tiles + kp
                                for t in range(t_blocks):
                                    nc.tensor.matmul(
                                        psums[t][:],
                                        wbin[:, k : k + k_step, ts(oi, P)],
                                        xbin[:, k : k + k_step, ts(t, t_tile)],
                                        start=(kb == 0 and kp == 0),
                                        stop=(
                                            kb == kb_blocks - 1
                                            and kp == kb_tiles - k_step
                                        ),
                                        perf_mode=perf,
                                    )
                        for t in range(t_blocks):
                            ob = evict_pool.tile(
                                [P, t_tile], mybir.dt.float32, name="ob"
                            )
                            # out = psum*4 + bias[p]  (exact; bias per-partition)
                            nc.vector.tensor_scalar(
                                ob[:],
                                psums[t][:],
                                4.0,
                                bias_sb[:, o : o + 1],
                                mult,
                                add,
                            )
                            nc.sync.dma_start(outT[ts(o, P), ts(t, t_tile)], ob[:])

    nc.compile()
    return nc


def build_nc_v5(
    t_loc: int = 2048,
    d_in: int = D_IN,
    d_out_loc: int = 2048,
    n_tile: int = 512,
    chunk_kt: int = 4,
    mm_dtype: mybir.dt = mybir.dt.float8e4,
    repeat: int = 1,
    mb_size: int = 4,
    kb_tiles: int = 8,
    xstage_bufs: int = 3,
    wstage_bufs: int = 3,
    # "full" | "dma_only" | "no_in" | "in_only" (loads+binarize, no outs/mm)
    # | "in_big" (pure big-DMA read stream) | "dma_sp" (dma_only, outs on SP)
    diag: str = "full",
):
    """v5: host-contiguous chunked layouts + per-mb-block x tiles + queue split.

    Inputs per core (host pre-arranged, layout-only transforms):
      x5 [MB, KG, 128, chunk_kt*n_tile] fp32 — x.T shard in (mb, kg) chunks:
         x5[mb, kg, p, kt*512 + c] = x[tok_base + mb*512 + c, kg*(chunk_kt*128)
         + kt*128 + p]
      w5 [NT, KG, 128, chunk_kt*512] fp32 — same chunking over W.T shard
      bias [128, d_out_loc] fp32 (host-replicated rows)
    Output: out [t_loc, d_out_loc] fp32 (same as v3).

    Differences vs v3:
      - all input DMAs are fully contiguous 1 MiB blocks (8 KiB rows)
      - x is cached per-mb-block in 4 separate tiles -> iteration i+1's
        x loads unblock as soon as column n3's (mb, n3) group retires
      - W binarize on ACT (Sign -> +-1), x on DVE (+-0.5); psum = S/2,
        evict = ACT Copy(scale=2) + DVE bias add
      - out DMAs + bias on the ACT HWDGE ring; input loads on the SP ring
      - DMA issue order: W n0 chunks, x mb0..3, W n1..n3
    """
    P_ = P
    assert t_loc % (mb_size * P_) == 0 and d_out_loc % n_tile == 0
    k_tiles = d_in // P_
    m_tiles = t_loc // P_
    n_tiles = d_out_loc // n_tile
    mb_blocks = m_tiles // mb_size
    assert k_tiles % chunk_kt == 0 and k_tiles % kb_tiles == 0
    kg_blocks = k_tiles // chunk_kt
    kb_blocks = k_tiles // kb_tiles
    chunk_f = chunk_kt * n_tile  # chunk free size (per partition elems)
    mblk = mb_size * P_
    assert mblk == n_tile, "x chunk layout assumes mb block width == n_tile"
    k_step = 2
    perf = mybir.MatmulPerfMode.DoubleRow

    nc = bacc.Bacc("TRN2", target_bir_lowering=False, debug=False)
    if diag == "in_big":
        x5 = nc.dram_tensor(
            "x5", [mb_blocks, P_, kg_blocks * chunk_f], mybir.dt.float32,
            kind="ExternalInput",
        )
        w5 = nc.dram_tensor(
            "w5", [n_tiles, P_, kg_blocks * chunk_f], mybir.dt.float32,
            kind="ExternalInput",
        )
    else:
        x5 = nc.dram_tensor(
            "x5", [mb_blocks, kg_blocks, P_, chunk_f], mybir.dt.float32,
            kind="ExternalInput",
        )
        w5 = nc.dram_tensor(
            "w5", [n_tiles, kg_blocks, P_, chunk_f], mybir.dt.float32,
            kind="ExternalInput",
        )
    bias = nc.dram_tensor("bias", [P_, d_out_loc], mybir.dt.float32,
                          kind="ExternalInput")
    out = nc.dram_tensor("out", [t_loc, d_out_loc], mybir.dt.float32,
                         kind="ExternalOutput")

    ge = mybir.AluOpType.is_ge
    sub = mybir.AluOpType.subtract
    add = mybir.AluOpType.add

    with tile.TileContext(nc) as tc:
        with (
            tc.tile_pool(name="const", bufs=1) as const_pool,
            tc.tile_pool(name="xbin_pool", bufs=mb_blocks) as xbin_pool,
            tc.tile_pool(name="xstage", bufs=xstage_bufs) as xstage_pool,
            tc.tile_pool(name="wstage", bufs=wstage_bufs) as wstage_pool,
            tc.tile_pool(name="wbin", bufs=2) as wbin_pool,
            tc.tile_pool(name="evict", bufs=8) as evict_pool,
            tc.tile_pool(name="psum", bufs=8, space="PSUM") as psum_pool,
        ):
            bias_sb = const_pool.tile([P_, d_out_loc], mybir.dt.float32,
                                      name="bias_sb")
            nc.scalar.dma_start(bias_sb[:], bias[:])

            xbin_fixed = wbin_fixed = ob_fixed = None
            if diag == "no_in":
                xbin_fixed = const_pool.tile(
                    [P_, k_tiles, mblk], mm_dtype, name="xbin_fixed")
                for _k in range(k_tiles):
                    nc.any.memset(xbin_fixed[:, _k, :], 0.5)
                wbin_fixed = const_pool.tile(
                    [P_, k_tiles, n_tile], mm_dtype, name="wbin_fixed")
                nc.any.memset(wbin_fixed[:], 1.0)
                dummy_x = const_pool.tile([P_, 16], mybir.dt.float32,
                                          name="dummy_x")
                nc.sync.dma_start(dummy_x[:], x5[0, 0, :P_, :16])
                dummy_w = const_pool.tile([P_, 16], mybir.dt.float32,
                                          name="dummy_w")
                nc.sync.dma_start(dummy_w[:], w5[0, 0, :P_, :16])
            if diag in ("dma_only", "dma_sp", "in_only", "in_big"):
                ob_fixed = const_pool.tile([P_, n_tile], mybir.dt.float32,
                                           name="ob_fixed")
                nc.any.memset(ob_fixed[:], 1.0)
                if diag in ("in_only", "in_big"):
                    # keep the output referenced
                    nc.scalar.dma_start(out[:P_, :n_tile], ob_fixed[:])

            import contextlib

            rep_ctx = (
                tc.For_i(0, repeat, 1) if repeat > 1 else contextlib.nullcontext()
            )
            with rep_ctx:
                # --- W column n0 loads first, then x mb0..3, then W n1..3 ---
                wbins = [None] * n_tiles
                xbins = [None] * mb_blocks

                def load_w_col(n):
                    wbin = wbin_pool.tile([P_, k_tiles, n_tile], mm_dtype,
                                          name="wbin")
                    for kg in range(kg_blocks):
                        wf = wstage_pool.tile(
                            [P_, chunk_kt, n_tile], mybir.dt.float32, name="wf")
                        nc.sync.dma_start(wf[:], w5[n, kg])
                        nc.scalar.sign(
                            wbin[:, kg * chunk_kt : (kg + 1) * chunk_kt, :],
                            wf[:],
                        )
                    return wbin

                def load_x_blk(mb):
                    xbin = xbin_pool.tile([P_, k_tiles, mblk], mm_dtype,
                                          name="xbin")
                    for kg in range(kg_blocks):
                        xf = xstage_pool.tile(
                            [P_, chunk_kt, mblk], mybir.dt.float32, name="xf")
                        nc.sync.dma_start(xf[:], x5[mb, kg])
                        nc.vector.tensor_scalar(
                            xbin[:, kg * chunk_kt : (kg + 1) * chunk_kt, :],
                            xf[:], 0.0, 0.5, ge, sub,
                        )
                    return xbin

                if diag == "in_big":
                    # pure read stream: 8 MiB contiguous DMAs, no compute
                    for mb in range(mb_blocks):
                        xf = xstage_pool.tile(
                            [P_, kg_blocks * chunk_f], mybir.dt.float32,
                            name="xbig")
                        nc.sync.dma_start(xf[:], x5[mb])
                    for n in range(n_tiles):
                        wf = wstage_pool.tile(
                            [P_, kg_blocks * chunk_f], mybir.dt.float32,
                            name="wbig")
                        nc.sync.dma_start(wf[:], w5[n])
                elif diag != "no_in":
                    wbins[0] = load_w_col(0)
                    for mb in range(mb_blocks):
                        xbins[mb] = load_x_blk(mb)
                    for n in range(1, n_tiles):
                        wbins[n] = load_w_col(n)
                else:
                    wbins = [wbin_fixed] * n_tiles
                    xbins = [xbin_fixed] * mb_blocks

                if diag in ("dma_only", "dma_sp"):
                    eng = nc.scalar if diag == "dma_only" else nc.sync
                    for n in range(n_tiles):
                        for m in range(m_tiles):
                            eng.dma_start(
                                out[ts(m, P_), ts(n, n_tile)], ob_fixed[:])
                elif diag in ("in_only", "in_big"):
                    pass
                else:
                    for n in range(n_tiles):
                        wbin = wbins[n]
                        for mb in range(mb_blocks):
                            xbin = xbins[mb]
                            psums = [
                                psum_pool.tile([P_, n_tile], mybir.dt.float32,
                                               name="psum")
                                for _ in range(mb_size)
                            ]
                            for kb in range(kb_blocks):
                                for mi in range(mb_size):
                                    for kp in range(0, kb_tiles, k_step):
                                        k = kb * kb_tiles + kp
                                        nc.tensor.matmul(
                                            psums[mi][:],
                                            xbin[:, k : k + k_step, ts(mi, P_)],
                                            wbin[:, k : k + k_step, :],
                                            start=(kb == 0 and kp == 0),
                                            stop=(kb == kb_blocks - 1
                                                  and kp == kb_tiles - k_step),
                                            perf_mode=perf,
                                        )
                            for mi in range(mb_size):
                                m = mb * mb_size + mi
                                t1 = evict_pool.tile([P_, n_tile],
                                                     mybir.dt.float32, name="t1")
                                # psum = S/2 (x +-0.5, W +-1) -> exact 2x
                                nc.scalar.activation(
                                    t1[:], psums[mi][:],
                                    mybir.ActivationFunctionType.Copy, scale=2.0)
                                ob = evict_pool.tile([P_, n_tile],
                                                     mybir.dt.float32, name="ob")
                                nc.vector.tensor_tensor(
                                    ob[:], t1[:], bias_sb[:, ts(n, n_tile)], add)
                                nc.scalar.dma_start(
                                    out[ts(m, P_), ts(n, n_tile)], ob[:])

    nc.compile()
    return nc


def build_nc_v6(
    t_loc: int = 2048,
    d_in: int = D_IN,
    d_out_loc: int = 2048,
    n_tile: int = 512,
    chunk_kt: int = 4,
    mm_dtype: mybir.dt = mybir.dt.float8e4,
    repeat: int = 1,
    mb_size: int = 4,
    kb_tiles: int = 8,
    xstage_bufs: int = 2,
    wstage_bufs: int = 2,
    out_on_act: bool = True,
    out_dtype: mybir.dt = mybir.dt.float32,
    interleave0: bool = False,
    evict_bufs: int = 8,
    ham_keepalive: bool = False,
    gsub: int | None = None,
):
    """v6: v5 layouts + rectangle-growth delivery/emission order.

    Load order W0,X0,W1,X1,W2,X2,W3,X3 (8 MiB units, 1 MiB chunks); groups
    (mb,n) are emitted as the rectangle grows so the PE stream tracks DMA
    arrival:  [W0,X0]:(0,0)  W1:(0,1)  X1:(1,0),(1,1)  W2:(0,2),(1,2)
    X2:(2,*)  W3:(0..2,3)  X3:(3,*).
    All four W columns stay resident (wbin bufs=4).  Binarize on DVE for
    both x and W (exact is_ge: +-0.5, psum=S/4, ACT Copy scale=4 + DVE
    bias add).  Out writes + bias on the ACT HWDGE ring; loads on SP.
    """
    P_ = P
    assert t_loc % (mb_size * P_) == 0 and d_out_loc % n_tile == 0
    k_tiles = d_in // P_
    m_tiles = t_loc // P_
    n_tiles = d_out_loc // n_tile
    mb_blocks = m_tiles // mb_size
    assert k_tiles % chunk_kt == 0 and k_tiles % kb_tiles == 0
    kg_blocks = k_tiles // chunk_kt
    kb_blocks = k_tiles // kb_tiles
    chunk_f = chunk_kt * n_tile
    mblk = mb_size * P_
    assert mblk == n_tile
    k_step = 2
    perf = mybir.MatmulPerfMode.DoubleRow

    nc = bacc.Bacc("TRN2", target_bir_lowering=False, debug=False)
    x5 = nc.dram_tensor(
        "x5", [mb_blocks, kg_blocks, P_, chunk_f], mybir.dt.float32,
        kind="ExternalInput",
    )
    w5 = nc.dram_tensor(
        "w5", [n_tiles, kg_blocks, P_, chunk_f], mybir.dt.float32,
        kind="ExternalInput",
    )
    bias = nc.dram_tensor("bias", [P_, d_out_loc], mybir.dt.float32,
                          kind="ExternalInput")
    out = nc.dram_tensor("out", [t_loc, d_out_loc], out_dtype,
                         kind="ExternalOutput")

    ge = mybir.AluOpType.is_ge
    sub = mybir.AluOpType.subtract
    add = mybir.AluOpType.add
    out_eng = nc.scalar if out_on_act else nc.sync

    with tile.TileContext(nc) as tc:
        with (
            tc.tile_pool(name="const", bufs=1) as const_pool,
            tc.tile_pool(name="xbin_pool", bufs=mb_blocks) as xbin_pool,
            tc.tile_pool(name="xstage", bufs=xstage_bufs) as xstage_pool,
            tc.tile_pool(name="wstage", bufs=wstage_bufs) as wstage_pool,
            tc.tile_pool(name="wbin", bufs=n_tiles) as wbin_pool,
            tc.tile_pool(name="evict", bufs=evict_bufs) as evict_pool,
            tc.tile_pool(name="psum", bufs=7 if ham_keepalive else 8,
                         space="PSUM") as psum_pool,
        ):
            import contextlib as _ctxlib

            _stk = _ctxlib.ExitStack()
            psum_scratch_pool = (
                _stk.enter_context(
                    tc.tile_pool(name="psum_ka", bufs=1, space="PSUM"))
                if ham_keepalive else None
            )
            bias_sb = const_pool.tile([P_, d_out_loc], mybir.dt.float32,
                                      name="bias_sb")
            nc.scalar.dma_start(bias_sb[:], bias[:])

            import contextlib

            rep_ctx = (
                tc.For_i(0, repeat, 1) if repeat > 1 else contextlib.nullcontext()
            )
            with rep_ctx:
                wbins = [None] * n_tiles
                xbins = [None] * mb_blocks

                # Tiny matmul tied to a freshly binarized chunk: keeps the
                # PE HAM window from seeing a fully idle 3.4us stretch
                # during the DMA-paced phase (else it re-throttles to
                # 1.2 GHz and every burst restarts cold).
                ka_psum = None
                if ham_keepalive:
                    ka_psum = psum_scratch_pool.tile(
                        [P_, 64], mybir.dt.float32, name="ka_psum")

                def keepalive(bin_tile, kg):
                    if not ham_keepalive:
                        return
                    k0 = kg * chunk_kt
                    nc.tensor.matmul(
                        ka_psum[:],
                        bin_tile[:, k0 : k0 + 2, :P_],
                        bin_tile[:, k0 : k0 + 2, :64],
                        start=True, stop=True,
                        perf_mode=perf, skip_group_check=True,
                    )

                def load_w_col(n):
                    wbin = wbin_pool.tile([P_, k_tiles, n_tile], mm_dtype,
                                          name="wbin")
                    for kg in range(kg_blocks):
                        wf = wstage_pool.tile(
                            [P_, chunk_kt, n_tile], mybir.dt.float32, name="wf")
                        nc.sync.dma_start(wf[:], w5[n, kg])
                        nc.vector.tensor_scalar(
                            wbin[:, kg * chunk_kt : (kg + 1) * chunk_kt, :],
                            wf[:], 0.0, 0.5, ge, sub,
                        )
                        keepalive(wbin, kg)
                    wbins[n] = wbin

                def load_x_blk(mb):
                    xbin = xbin_pool.tile([P_, k_tiles, mblk], mm_dtype,
                                          name="xbin")
                    for kg in range(kg_blocks):
                        xf = xstage_pool.tile(
                            [P_, chunk_kt, mblk], mybir.dt.float32, name="xf")
                        nc.sync.dma_start(xf[:], x5[mb, kg])
                        nc.vector.tensor_scalar(
                            xbin[:, kg * chunk_kt : (kg + 1) * chunk_kt, :],
                            xf[:], 0.0, 0.5, ge, sub,
                        )
                        keepalive(xbin, kg)
                    xbins[mb] = xbin

                def group(mb, n):
                    g = gsub or mb_size
                    for sub in range(mb_size // g):
                        _subgroup(mb, n, sub * g, g)

                def _subgroup(mb, n, mi0, g):
                    xbin, wbin = xbins[mb], wbins[n]
                    psums = [
                        psum_pool.tile([P_, n_tile], mybir.dt.float32,
                                       name="psum")
                        for _ in range(g)
                    ]
                    for kb in range(kb_blocks):
                        for mi_l in range(g):
                            for kp in range(0, kb_tiles, k_step):
                                k = kb * kb_tiles + kp
                                nc.tensor.matmul(
                                    psums[mi_l][:],
                                    xbin[:, k : k + k_step,
                                         ts(mi0 + mi_l, P_)],
                                    wbin[:, k : k + k_step, :],
                                    start=(kb == 0 and kp == 0),
                                    stop=(kb == kb_blocks - 1
                                          and kp == kb_tiles - k_step),
                                    perf_mode=perf,
                                )
                    for mi_l in range(g):
                        mi = mi0 + mi_l
                        m = mb * mb_size + mi
                        t1 = evict_pool.tile([P_, n_tile], mybir.dt.float32,
                                             name="t1")
                        # psum = S/4 (both operands +-0.5) -> exact 4x
                        nc.scalar.activation(
                            t1[:], psums[mi_l][:],
                            mybir.ActivationFunctionType.Copy, scale=4.0)
                        ob = evict_pool.tile([P_, n_tile], out_dtype,
                                             name="ob")
                        nc.vector.tensor_tensor(
                            ob[:], t1[:], bias_sb[:, ts(n, n_tile)], add)
                        out_eng.dma_start(out[ts(m, P_), ts(n, n_tile)], ob[:])

                # rectangle growth: alternate W cols and x blocks.
                # Step 0 interleaves W0/X0 chunks so group (0,0) is k-paced
                # from the first ~2 MiB of arrivals.
                if interleave0:
                    wbin0 = wbin_pool.tile([P_, k_tiles, n_tile], mm_dtype,
                                           name="wbin")
                    xbin0 = xbin_pool.tile([P_, k_tiles, mblk], mm_dtype,
                                           name="xbin")
                    for kg in range(kg_blocks):
                        wf = wstage_pool.tile(
                            [P_, chunk_kt, n_tile], mybir.dt.float32, name="wf")
                        nc.sync.dma_start(wf[:], w5[0, kg])
                        xf = xstage_pool.tile(
                            [P_, chunk_kt, mblk], mybir.dt.float32, name="xf")
                        nc.sync.dma_start(xf[:], x5[0, kg])
                        nc.vector.tensor_scalar(
                            wbin0[:, kg * chunk_kt : (kg + 1) * chunk_kt, :],
                            wf[:], 0.0, 0.5, ge, sub,
                        )
                        keepalive(wbin0, kg)
                        nc.vector.tensor_scalar(
                            xbin0[:, kg * chunk_kt : (kg + 1) * chunk_kt, :],
                            xf[:], 0.0, 0.5, ge, sub,
                        )
                        keepalive(xbin0, kg)
                    wbins[0] = wbin0
                    xbins[0] = xbin0
                else:
                    load_w_col(0)
                    load_x_blk(0)
                group(0, 0)
                for s in range(1, n_tiles + mb_blocks - 1):
                    if s % 2 == 1:  # new W column
                        n = (s + 1) // 2
                        load_w_col(n)
                        for mb in range((s + 1) // 2):
                            group(mb, n)
                    else:  # new x block
                        mb = s // 2
                        load_x_blk(mb)
                        for n in range(s // 2 + 1):
                            group(mb, n)

            _stk.close()

    nc.compile()
    return nc


def make_in_maps_v5(x, fp_weight, fp_bias, chunk_kt: int = 4):
    """Host-side sharding + relayout (layout only: transpose/reshape/slice)."""
    xT = np.asarray(x, dtype=np.float32).T  # [D_IN, N_TOK]
    wT = np.asarray(fp_weight, dtype=np.float32).T  # [D_IN, D_OUT]
    bias = np.asarray(fp_bias, dtype=np.float32)
    kg_blocks = D_IN // (chunk_kt * P)
    in_maps = []
    for c in range(N_CORES):
        i, j = divmod(c, O_GRP)
        xs = xT[:, i * T_LOC : (i + 1) * T_LOC]  # [4096, 2048]
        ws = wT[:, j * O_LOC : (j + 1) * O_LOC]  # [4096, 2048]
        # [kg, kt, p, blk, c] -> [blk, kg, p, kt, c]
        x5 = np.ascontiguousarray(
            xs.reshape(kg_blocks, chunk_kt, P, 4, 512).transpose(3, 0, 2, 1, 4)
        ).reshape(4, kg_blocks, P, chunk_kt * 512)
        w5 = np.ascontiguousarray(
            ws.reshape(kg_blocks, chunk_kt, P, 4, 512).transpose(3, 0, 2, 1, 4)
        ).reshape(4, kg_blocks, P, chunk_kt * 512)
        in_maps.append(
            {
                "x5": x5,
                "w5": w5,
                "bias": np.ascontiguousarray(
                    np.broadcast_to(
                        bias[None, j * O_LOC : (j + 1) * O_LOC], (P, O_LOC)
                    )
                ),
            }
        )
    return in_maps


_NC_CACHE: dict = {}

# production sharding: 4-way tokens x 2-way out-features
T_GRP, O_GRP = 4, 2
T_LOC = N_TOK // T_GRP  # 2048
O_LOC = D_OUT // O_GRP  # 2048


# Production build: v6 rect-growth schedule, fp16 out on the ACT ring,
# interleaved step-0 loads, deep stage pipelining.  PROD_KW is shared by
# kernel() and test.py's repeat-loop timing builds.
PROD_CHUNK_KT = 2
PROD_KW = dict(
    chunk_kt=PROD_CHUNK_KT,
    xstage_bufs=4,
    wstage_bufs=4,
    evict_bufs=8,
    out_dtype=mybir.dt.float16,
    out_on_act=True,
    interleave0=True,
    ham_keepalive=False,
)


def build_production(repeat: int = 1):
    return build_nc_v6(repeat=repeat, **PROD_KW)


def make_in_maps_production(x, fp_weight, fp_bias):
    return make_in_maps_v5(x, fp_weight, fp_bias, chunk_kt=PROD_CHUNK_KT)


def _get_nc(key=("v6",)):
    if key not in _NC_CACHE:
        _NC_CACHE[key] = build_production()
    return _NC_CACHE[key]


def make_in_maps(x, fp_weight, fp_bias):
    """Host-side sharding (layout only: transpose + slice + replicate)."""
    xT = np.ascontiguousarray(np.asarray(x, dtype=np.float32).T)  # [D_IN, N_TOK]
    wT = np.ascontiguousarray(np.asarray(fp_weight, dtype=np.float32).T)
    bias = np.asarray(fp_bias, dtype=np.float32)
    in_maps = []
    for c in range(N_CORES):
        i, j = divmod(c, O_GRP)
        in_maps.append(
            {
                "xT": np.ascontiguousarray(xT[:, i * T_LOC : (i + 1) * T_LOC]),
                "wT": np.ascontiguousarray(wT[:, j * O_LOC : (j + 1) * O_LOC]),
                "bias": np.ascontiguousarray(
                    np.broadcast_to(
                        bias[None, j * O_LOC : (j + 1) * O_LOC], (P, O_LOC)
                    )
                ),
            }
        )
    return in_maps


def assemble(results) -> np.ndarray:
    out = np.empty((N_TOK, D_OUT), np.float32)
    for c in range(N_CORES):
        i, j = divmod(c, O_GRP)
        out[i * T_LOC : (i + 1) * T_LOC, j * O_LOC : (j + 1) * O_LOC] = results[c][
            "out"
        ]
    return out


def kernel(x: np.ndarray, fp_weight: np.ndarray, fp_bias: np.ndarray) -> np.ndarray:
    assert x.shape == (N_TOK, D_IN) and fp_weight.shape == (D_OUT, D_IN)
    from concourse.bass_utils import run_bass_kernel_spmd

    nc = _get_nc()
    in_maps = make_in_maps_production(x, fp_weight, fp_bias)
    res = run_bass_kernel_spmd(nc, in_maps, core_ids=list(range(N_CORES)))
    return assemble(res.results)



# revision 30
# speedup vs baseline: 1.1947x; 1.0592x over previous
"""BinarizedLinear Trainium2 kernel.

out = sign(x) @ sign(W).T + bias,  x:[8192,4096] W:[4096,4096] bias:[4096] (fp32)

Production config (build_nc_v6): 8 cores as a 4x2 grid — tokens split 4
ways (2048 rows/core), out-features split 2 ways (2048 cols/core), which
minimizes per-core HBM traffic (72 MiB: x 32 + W 32 + out-fp16 8).

Per core (v6):
  - Host relayouts x.T / W.T shards (pure layout: transpose/reshape/
    slice; all math on device) into [blk, kg, 128, chunk] fp32 so every
    input DMA is a fully contiguous 0.5-1 MiB block — measured read
    stream hits ~361 GB/s (HBM-per-NC cap ~358).
  - Rectangle-growth delivery/emission: load W-col0 + x-blk0 chunk-
    interleaved, then alternate new W columns and x blocks; matmul
    groups (mb, n) are emitted exactly as the growing W-cols x x-blks
    rectangle enables them, so the in-order PE stream tracks DMA
    arrival and the serial ramp is minimized.  All 4 W columns stay
    resident (fp8, 4 MiB); x is cached per-mb-block in 4 tiles.
  - Binarize to +-0.5 in fp8e4 on DVE (is_ge -> {0,1} - 0.5; exact,
    including sign(0)=+1); 3-4-deep stage pools keep the read pipeline
    full (2-deep stalls it on the ~2us DMA-completion latency).
  - Matmuls run fp8 DoubleRow (K=256/instr, N=512, ~227 ns/MM measured;
    PE+evict floor ~232 us for the 1024-MM stream).
  - PSUM holds S/4 exactly (S = integer +-1 dot product); eviction is
    ACT Copy(scale=4) + DVE bias add, written as fp16 on the ACT HWDGE
    ring (host upcasts; |S| << 2048 so fp16 keeps S exact and only
    rounds the small bias: rel err ~1e-4).  Writes measured ~200 GB/s,
    so halving write bytes matters; loads stay on the SP ring to avoid
    head-of-line blocking.

Component floors (R-slope, per iteration): reads+binarize 186 us,
PE+evict+writes 232 us.  Production measured 305-340 us depending on
device thermal state (was 363-381 us for the v3 baseline under the same
conditions); remainder over the floors is the DMA-paced ramp (first
~16 MiB before the first group can retire) plus read/write turnaround.
Note: identical NEFFs drift +30 us over ~20 min of sustained benching —
always A/B within one process and distrust cross-process deltas < 20 us.
"""

import os
import sys

sys.path.insert(0, "/opt/trn_rl_repo")

import numpy as np

import concourse.bass as bass
import concourse.mybir as mybir
import concourse.tile as tile
from concourse import bacc
from concourse.bass import ts

N_CORES = 8
P = 128

# Full problem shapes (hardcoded per contract)
N_TOK, D_IN, D_OUT = 8192, 4096, 4096


def build_nc(
    t_loc: int = N_TOK // N_CORES,
    d_in: int = D_IN,
    d_out: int = D_OUT,
    n_tile: int = 512,
    mm_dtype: mybir.dt = mybir.dt.bfloat16,
    double_row: bool = False,
    repeat: int = 1,
    n_pair: int = 1,
):
    """Build the per-core Bass program.

    Inputs (per core):
      xT   [d_in, t_loc] fp32   (x.T shard)
      wT   [d_in, d_out] fp32   (W.T, full)
      bias [128, d_out]  fp32   (host-replicated rows)
    Output:
      out  [t_loc, d_out] fp32
    """
    assert t_loc % P == 0 and d_in % P == 0 and d_out % n_tile == 0
    k_tiles = d_in // P
    m_tiles = t_loc // P
    n_tiles = d_out // n_tile
    if double_row:
        assert mm_dtype in (mybir.dt.float8e4, mybir.dt.float8e5)
        assert k_tiles % 2 == 0

    nc = bacc.Bacc("TRN2", target_bir_lowering=False, debug=False)

    xT = nc.dram_tensor("xT", [d_in, t_loc], mybir.dt.float32, kind="ExternalInput")
    wT = nc.dram_tensor("wT", [d_in, d_out], mybir.dt.float32, kind="ExternalInput")
    bias = nc.dram_tensor("bias", [P, d_out], mybir.dt.float32, kind="ExternalInput")
    out = nc.dram_tensor("out", [t_loc, d_out], mybir.dt.float32, kind="ExternalOutput")

    ge = mybir.AluOpType.is_ge
    sub = mybir.AluOpType.subtract
    add = mybir.AluOpType.add

    with tile.TileContext(nc) as tc:
        with (
            tc.tile_pool(name="const", bufs=1) as const_pool,
            tc.tile_pool(name="xbin_pool", bufs=1) as xbin_pool,
            tc.tile_pool(name="xstage", bufs=3) as xstage_pool,
            tc.tile_pool(name="wstage", bufs=6) as wstage_pool,
            tc.tile_pool(name="wbin", bufs=2 * n_pair) as wbin_pool,
            tc.tile_pool(name="evict", bufs=8) as evict_pool,
            tc.tile_pool(name="psum", bufs=6, space="PSUM") as psum_pool,
        ):
            # bias, replicated on host to [P, d_out]
            bias_sb = const_pool.tile([P, d_out], mybir.dt.float32, name="bias_sb")
            nc.sync.dma_start(bias_sb[:], bias[:])

            import contextlib

            rep_ctx = (
                tc.For_i(0, repeat, 1) if repeat > 1 else contextlib.nullcontext()
            )
            with rep_ctx:
                _body(
                    nc, tc, xT, wT, out, bias_sb,
                    xbin_pool, xstage_pool, wstage_pool, wbin_pool, evict_pool,
                    psum_pool, t_loc, n_tile, k_tiles, m_tiles, n_tiles,
                    mm_dtype, double_row, n_pair,
                )

    nc.compile()
    return nc


def _body(
    nc, tc, xT, wT, out, bias_sb,
    xbin_pool, xstage_pool, wstage_pool, wbin_pool, evict_pool,
    psum_pool, t_loc, n_tile, k_tiles, m_tiles, n_tiles,
    mm_dtype, double_row, n_pair=1,
):
    ge = mybir.AluOpType.is_ge
    sub = mybir.AluOpType.subtract
    add = mybir.AluOpType.add
    assert n_tiles % n_pair == 0

    # ---- load + binarize all of x.T into SBUF (mm_dtype, +-0.5) ----
    xbin = xbin_pool.tile([P, k_tiles, t_loc], mm_dtype, name="xbin")
    for k in range(k_tiles):
        xf = xstage_pool.tile([P, t_loc], mybir.dt.float32, name="xf")
        nc.sync.dma_start(xf[:], xT[ts(k, P), :])
        nc.vector.tensor_scalar(xbin[:, k, :], xf[:], 0.0, 0.5, ge, sub)

    # ---- stream W.T by group of n_pair n-tiles; matmul; evict ----
    for ng in range(n_tiles // n_pair):
        wbins = []
        for j in range(n_pair):
            n = ng * n_pair + j
            wbin = wbin_pool.tile([P, k_tiles, n_tile], mm_dtype, name="wbin")
            for k in range(k_tiles):
                wf = wstage_pool.tile([P, n_tile], mybir.dt.float32, name="wf")
                nc.sync.dma_start(wf[:], wT[ts(k, P), ts(n, n_tile)])
                nc.vector.tensor_scalar(wbin[:, k, :], wf[:], 0.0, 0.5, ge, sub)
            wbins.append(wbin)

        for m in range(m_tiles):
            psums = [
                psum_pool.tile([P, n_tile], mybir.dt.float32, name="psum")
                for _ in range(n_pair)
            ]
            if double_row:
                for k in range(0, k_tiles, 2):
                    for j in range(n_pair):
                        nc.tensor.matmul(
                            psums[j][:],
                            xbin[:, k : k + 2, ts(m, P)],
                            wbins[j][:, k : k + 2, :],
                            start=(k == 0),
                            stop=(k == k_tiles - 2),
                            perf_mode=mybir.MatmulPerfMode.DoubleRow,
                        )
            else:
                for k in range(k_tiles):
                    for j in range(n_pair):
                        nc.tensor.matmul(
                            psums[j][:],
                            xbin[:, k, ts(m, P)],
                            wbins[j][:, k, :],
                            start=(k == 0),
                            stop=(k == k_tiles - 1),
                        )
            for j in range(n_pair):
                n = ng * n_pair + j
                # out = 4 * psum + bias   (exact: psum = S/4, S integer)
                t1 = evict_pool.tile([P, n_tile], mybir.dt.float32, name="t1")
                nc.scalar.activation(
                    t1[:], psums[j][:], mybir.ActivationFunctionType.Copy, scale=4.0
                )
                ob = evict_pool.tile([P, n_tile], mybir.dt.float32, name="ob")
                nc.vector.tensor_tensor(
                    ob[:], t1[:], bias_sb[:, ts(n, n_tile)], add
                )
                nc.sync.dma_start(out[ts(m, P), ts(n, n_tile)], ob[:])


def build_nc_v3(
    t_loc: int = 2048,
    d_in: int = D_IN,
    d_out_loc: int = 2048,
    n_tile: int = 512,
    mm_dtype: mybir.dt = mybir.dt.float8e4,
    double_row: bool = True,
    repeat: int = 1,
    mb_size: int = 4,
    kb_tiles: int = 8,
    x_m_major: bool = False,
    w_sign_act: bool = False,
    xstage_bufs: int = 3,
    wstage_bufs: int = 6,
    diag: str = "full",  # "full" | "dma_only" | "no_in" (timing diagnostics)
):
    """4x2-sharded variant: tokens split 4 ways, out-features 2 ways.

    Per core: xT [d_in, t_loc], wT [d_in, d_out_loc], bias [P, d_out_loc],
    out [t_loc, d_out_loc].  x binarized+cached in SBUF; W streamed by
    n-tile.  Matmuls are emitted kb-block-major so the in-order PE stream
    tracks DMA arrival order during the ramp.
    """
    assert t_loc % P == 0 and d_in % P == 0 and d_out_loc % n_tile == 0
    k_tiles = d_in // P
    m_tiles = t_loc // P
    n_tiles = d_out_loc // n_tile
    assert m_tiles % mb_size == 0 and k_tiles % kb_tiles == 0
    if double_row:
        assert kb_tiles % 2 == 0

    nc = bacc.Bacc("TRN2", target_bir_lowering=False, debug=False)
    xT = nc.dram_tensor("xT", [d_in, t_loc], mybir.dt.float32, kind="ExternalInput")
    wT = nc.dram_tensor("wT", [d_in, d_out_loc], mybir.dt.float32, kind="ExternalInput")
    bias = nc.dram_tensor("bias", [P, d_out_loc], mybir.dt.float32, kind="ExternalInput")
    out = nc.dram_tensor(
        "out", [t_loc, d_out_loc], mybir.dt.float32, kind="ExternalOutput"
    )

    ge = mybir.AluOpType.is_ge
    sub = mybir.AluOpType.subtract
    add = mybir.AluOpType.add
    k_step = 2 if double_row else 1
    perf = mybir.MatmulPerfMode.DoubleRow if double_row else None
    kb_blocks = k_tiles // kb_tiles

    with tile.TileContext(nc) as tc:
        with (
            tc.tile_pool(name="const", bufs=1) as const_pool,
            tc.tile_pool(name="xbin_pool", bufs=1) as xbin_pool,
            tc.tile_pool(name="xstage", bufs=xstage_bufs) as xstage_pool,
            tc.tile_pool(name="wstage", bufs=wstage_bufs) as wstage_pool,
            tc.tile_pool(name="wbin", bufs=2) as wbin_pool,
            tc.tile_pool(name="evict", bufs=8) as evict_pool,
            tc.tile_pool(name="psum", bufs=8, space="PSUM") as psum_pool,
        ):
            bias_sb = const_pool.tile([P, d_out_loc], mybir.dt.float32, name="bias_sb")
            nc.sync.dma_start(bias_sb[:], bias[:])

            # Diagnostic-only fixed tiles (outside the timed repeat loop).
            xbin_fixed = wbin_fixed = ob_fixed = None
            if diag == "no_in":
                xbin_fixed = const_pool.tile(
                    [P, k_tiles, t_loc], mm_dtype, name="xbin_fixed"
                )
                for _k in range(k_tiles):
                    nc.any.memset(xbin_fixed[:, _k, :], 0.5)
                wbin_fixed = const_pool.tile(
                    [P, k_tiles, n_tile], mm_dtype, name="wbin_fixed"
                )
                nc.any.memset(wbin_fixed[:], 0.5)
                dummy_x = const_pool.tile([P, 16], mybir.dt.float32, name="dummy_x")
                nc.sync.dma_start(dummy_x[:], xT[:P, :16])
                dummy_w = const_pool.tile([P, 16], mybir.dt.float32, name="dummy_w")
                nc.sync.dma_start(dummy_w[:], wT[:P, :16])
            if diag == "dma_only":
                ob_fixed = const_pool.tile([P, n_tile], mybir.dt.float32, name="ob_fixed")
                nc.any.memset(ob_fixed[:], 1.0)

            import contextlib

            rep_ctx = (
                tc.For_i(0, repeat, 1) if repeat > 1 else contextlib.nullcontext()
            )
            with rep_ctx:
                xbin = (
                    xbin_fixed
                    if diag == "no_in"
                    else xbin_pool.tile([P, k_tiles, t_loc], mm_dtype, name="xbin")
                )
                if diag == "no_in":
                    pass
                elif x_m_major:
                    # Load x by token-block (all k per block) so the first
                    # matmul group's operands arrive early — matches the
                    # (mb, kb) consumption order of the in-order PE stream.
                    mblk = mb_size * P
                    for mb0 in range(t_loc // mblk):
                        for k in range(k_tiles):
                            xf = xstage_pool.tile(
                                [P, mblk], mybir.dt.float32, name="xf"
                            )
                            nc.sync.dma_start(
                                xf[:], xT[ts(k, P), ts(mb0, mblk)]
                            )
                            nc.vector.tensor_scalar(
                                xbin[:, k, ts(mb0, mblk)], xf[:], 0.0, 0.5, ge, sub
                            )
                else:
                    for k in range(k_tiles):
                        xf = xstage_pool.tile(
                            [P, t_loc], mybir.dt.float32, name="xf"
                        )
                        nc.sync.dma_start(xf[:], xT[ts(k, P), :])
                        nc.vector.tensor_scalar(
                            xbin[:, k, :], xf[:], 0.0, 0.5, ge, sub
                        )

                for n in range(n_tiles):
                    wbin = (
                        wbin_fixed
                        if diag == "no_in"
                        else wbin_pool.tile([P, k_tiles, n_tile], mm_dtype, name="wbin")
                    )
                    if diag != "no_in":
                        for k in range(k_tiles):
                            wf = wstage_pool.tile(
                                [P, n_tile], mybir.dt.float32, name="wf"
                            )
                            nc.sync.dma_start(wf[:], wT[ts(k, P), ts(n, n_tile)])
                            if w_sign_act:
                                nc.scalar.sign(wbin[:, k, :], wf[:])
                            else:
                                nc.vector.tensor_scalar(
                                    wbin[:, k, :], wf[:], 0.0, 0.5, ge, sub
                                )

                    if diag == "dma_only":
                        for mb in range(m_tiles // mb_size):
                            for mi in range(mb_size):
                                m = mb * mb_size + mi
                                nc.sync.dma_start(
                                    out[ts(m, P), ts(n, n_tile)], ob_fixed[:]
                                )
                        continue

                    for mb in range(m_tiles // mb_size):
                        psums = [
                            psum_pool.tile(
                                [P, n_tile], mybir.dt.float32, name="psum"
                            )
                            for _ in range(mb_size)
                        ]
                        for kb in range(kb_blocks):
                            for mi in range(mb_size):
                                m = mb * mb_size + mi
                                for kp in range(0, kb_tiles, k_step):
                                    k = kb * kb_tiles + kp
                                    nc.tensor.matmul(
                                        psums[mi][:],
                                        xbin[:, k : k + k_step, ts(m, P)],
                                        wbin[:, k : k + k_step, :],
                                        start=(kb == 0 and kp == 0),
                                        stop=(
                                            kb == kb_blocks - 1
                                            and kp == kb_tiles - k_step
                                        ),
                                        perf_mode=perf,
                                    )
                        for mi in range(mb_size):
                            m = mb * mb_size + mi
                            t1 = evict_pool.tile(
                                [P, n_tile], mybir.dt.float32, name="t1"
                            )
                            if w_sign_act:
                                # ACT is busy with W Sign; scale on DVE
                                nc.vector.tensor_scalar_mul(
                                    t1[:], psums[mi][:], 2.0
                                )
                            else:
                                nc.scalar.activation(
                                    t1[:],
                                    psums[mi][:],
                                    mybir.ActivationFunctionType.Copy,
                                    scale=4.0,
                                )
                            ob = evict_pool.tile(
                                [P, n_tile], mybir.dt.float32, name="ob"
                            )
                            nc.vector.tensor_tensor(
                                ob[:], t1[:], bias_sb[:, ts(n, n_tile)], add
                            )
                            nc.sync.dma_start(out[ts(m, P), ts(n, n_tile)], ob[:])

    nc.compile()
    return nc


def build_nc_v4(
    t_loc: int = 2048,
    d_in: int = D_IN,
    d_out_loc: int = 2048,
    o_group: int = 512,
    t_tile: int = 512,
    mm_dtype: mybir.dt = mybir.dt.float8e4,
    double_row: bool = True,
    repeat: int = 1,
    kb_tiles: int = 8,
    diag: str = "full",  # "full" | "no_w" | "no_x" | "no_bin" (timing diagnostics)
):
    """out.T orientation: W chunks are the stationary operand, x moving.

    Each LDWEIGHTS (wbin [128, 2, 128]) serves t_loc/t_tile consecutive
    matmuls (moving over token tiles), hiding the DoubleRow weight-load
    cost.  PSUM is [outf, tokens]; eviction is a single DVE
    tensor_scalar(psum*4 + bias[p]) with per-partition bias AP; output is
    written as out.T [d_out_loc, t_loc] and un-transposed on host.

    Inputs per core: xT [d_in, t_loc], wT [d_in, d_out_loc],
    biasT [P, d_out_loc//P] (bias.reshape(-1, P).T), outT [d_out_loc, t_loc].
    """
    assert t_loc % t_tile == 0 and d_in % P == 0 and d_out_loc % o_group == 0
    k_tiles = d_in // P
    o_blocks = d_out_loc // P
    o_per_g = o_group // P
    t_blocks = t_loc // t_tile
    kb_blocks = k_tiles // kb_tiles
    if double_row:
        assert kb_tiles % 2 == 0
    k_step = 2 if double_row else 1
    perf = mybir.MatmulPerfMode.DoubleRow if double_row else None

    nc = bacc.Bacc("TRN2", target_bir_lowering=False, debug=False)
    xT = nc.dram_tensor("xT", [d_in, t_loc], mybir.dt.float32, kind="ExternalInput")
    wT = nc.dram_tensor("wT", [d_in, d_out_loc], mybir.dt.float32, kind="ExternalInput")
    biasT = nc.dram_tensor(
        "biasT", [P, o_blocks], mybir.dt.float32, kind="ExternalInput"
    )
    outT = nc.dram_tensor(
        "outT", [d_out_loc, t_loc], mybir.dt.float32, kind="ExternalOutput"
    )

    ge = mybir.AluOpType.is_ge
    sub = mybir.AluOpType.subtract
    mult = mybir.AluOpType.mult
    add = mybir.AluOpType.add

    with tile.TileContext(nc) as tc:
        with (
            tc.tile_pool(name="const", bufs=1) as const_pool,
            tc.tile_pool(name="xbin_pool", bufs=1) as xbin_pool,
            tc.tile_pool(name="xstage", bufs=3) as xstage_pool,
            tc.tile_pool(name="wstage", bufs=6) as wstage_pool,
            tc.tile_pool(name="wbin", bufs=2) as wbin_pool,
            tc.tile_pool(name="evict", bufs=8) as evict_pool,
            tc.tile_pool(name="psum", bufs=8, space="PSUM") as psum_pool,
        ):
            bias_sb = const_pool.tile([P, o_blocks], mybir.dt.float32, name="bias_sb")
            nc.sync.dma_start(bias_sb[:], biasT[:])

            # Diagnostic-only: pre-filled operand tiles living outside the
            # timed repeat loop.
            xbin_fixed = wbin_fixed = None
            if diag in ("no_x", "no_bin", "pe_only"):
                xbin_fixed = const_pool.tile(
                    [P, k_tiles, t_loc], mm_dtype, name="xbin_fixed"
                )
                for _k in range(k_tiles):
                    nc.any.memset(xbin_fixed[:, _k, :], 0.5)
            if diag in ("no_w", "no_bin", "pe_only"):
                wbin_fixed = const_pool.tile(
                    [P, k_tiles, o_group], mm_dtype, name="wbin_fixed"
                )
                nc.any.memset(wbin_fixed[:], 0.5)

            # keep skipped inputs referenced so walrus accepts the NEFF
            if diag in ("no_x", "pe_only"):
                dummy_x = const_pool.tile([P, 16], mybir.dt.float32, name="dummy_x")
                nc.sync.dma_start(dummy_x[:], xT[:P, :16])
            if diag in ("no_w", "pe_only"):
                dummy_w = const_pool.tile([P, 16], mybir.dt.float32, name="dummy_w")
                nc.sync.dma_start(dummy_w[:], wT[:P, :16])

            import contextlib

            rep_ctx = (
                tc.For_i(0, repeat, 1) if repeat > 1 else contextlib.nullcontext()
            )
            with rep_ctx:
                if xbin_fixed is not None:
                    xbin = xbin_fixed
                else:
                    xbin = xbin_pool.tile([P, k_tiles, t_loc], mm_dtype, name="xbin")
                if diag not in ("no_x", "pe_only"):
                    for k in range(k_tiles):
                        xf = xstage_pool.tile(
                            [P, t_loc], mybir.dt.float32, name="xf"
                        )
                        nc.sync.dma_start(xf[:], xT[ts(k, P), :])
                        if diag != "no_bin":
                            nc.vector.tensor_scalar(
                                xbin[:, k, :], xf[:], 0.0, 0.5, ge, sub
                            )

                for og in range(d_out_loc // o_group):
                    if wbin_fixed is not None:
                        wbin = wbin_fixed
                    else:
                        wbin = wbin_pool.tile(
                            [P, k_tiles, o_group], mm_dtype, name="wbin"
                        )
                    if diag not in ("no_w", "pe_only"):
                        for k in range(k_tiles):
                            wf = wstage_pool.tile(
                                [P, o_group], mybir.dt.float32, name="wf"
                            )
                            nc.sync.dma_start(wf[:], wT[ts(k, P), ts(og, o_group)])
                            if diag != "no_bin":
                                nc.vector.tensor_scalar(
                                    wbin[:, k, :], wf[:], 0.0, 0.5, ge, sub
                                )

                    for oi in range(o_per_g):
                        o = og * o_per_g + oi
                        psums = [
                            psum_pool.tile([P, t_tile], mybir.dt.float32, name="psum")
                            for _ in range(t_blocks)
                        ]
                        for kb in range(kb_blocks):
                            for kp in range(0, kb_tiles, k_step):
                                k = kb * kb_tiles + kp
                                for t in range(t_blocks):
                                    nc.tensor.matmul(
                                        psums[t][:],
                                        wbin[:, k : k + k_step, ts(oi, P)],
                                        xbin[:, k : k + k_step, ts(t, t_tile)],
                                        start=(kb == 0 and kp == 0),
                                        stop=(
                                            kb == kb_blocks - 1
                                            and kp == kb_tiles - k_step
                                        ),
                                        perf_mode=perf,
                                    )
                        for t in range(t_blocks):
                            ob = evict_pool.tile(
                                [P, t_tile], mybir.dt.float32, name="ob"
                            )
                            # out = psum*4 + bias[p]  (exact; bias per-partition)
                            nc.vector.tensor_scalar(
                                ob[:],
                                psums[t][:],
                                4.0,
                                bias_sb[:, o : o + 1],
                                mult,
                                add,
                            )
                            nc.sync.dma_start(outT[ts(o, P), ts(t, t_tile)], ob[:])

    nc.compile()
    return nc


def build_nc_v5(
    t_loc: int = 2048,
    d_in: int = D_IN,
    d_out_loc: int = 2048,
    n_tile: int = 512,
    chunk_kt: int = 4,
    mm_dtype: mybir.dt = mybir.dt.float8e4,
    repeat: int = 1,
    mb_size: int = 4,
    kb_tiles: int = 8,
    xstage_bufs: int = 3,
    wstage_bufs: int = 3,
    # "full" | "dma_only" | "no_in" | "in_only" (loads+binarize, no outs/mm)
    # | "in_big" (pure big-DMA read stream) | "dma_sp" (dma_only, outs on SP)
    diag: str = "full",
):
    """v5: host-contiguous chunked layouts + per-mb-block x tiles + queue split.

    Inputs per core (host pre-arranged, layout-only transforms):
      x5 [MB, KG, 128, chunk_kt*n_tile] fp32 — x.T shard in (mb, kg) chunks:
         x5[mb, kg, p, kt*512 + c] = x[tok_base + mb*512 + c, kg*(chunk_kt*128)
         + kt*128 + p]
      w5 [NT, KG, 128, chunk_kt*512] fp32 — same chunking over W.T shard
      bias [128, d_out_loc] fp32 (host-replicated rows)
    Output: out [t_loc, d_out_loc] fp32 (same as v3).

    Differences vs v3:
      - all input DMAs are fully contiguous 1 MiB blocks (8 KiB rows)
      - x is cached per-mb-block in 4 separate tiles -> iteration i+1's
        x loads unblock as soon as column n3's (mb, n3) group retires
      - W binarize on ACT (Sign -> +-1), x on DVE (+-0.5); psum = S/2,
        evict = ACT Copy(scale=2) + DVE bias add
      - out DMAs + bias on the ACT HWDGE ring; input loads on the SP ring
      - DMA issue order: W n0 chunks, x mb0..3, W n1..n3
    """
    P_ = P
    assert t_loc % (mb_size * P_) == 0 and d_out_loc % n_tile == 0
    k_tiles = d_in // P_
    m_tiles = t_loc // P_
    n_tiles = d_out_loc // n_tile
    mb_blocks = m_tiles // mb_size
    assert k_tiles % chunk_kt == 0 and k_tiles % kb_tiles == 0
    kg_blocks = k_tiles // chunk_kt
    kb_blocks = k_tiles // kb_tiles
    chunk_f = chunk_kt * n_tile  # chunk free size (per partition elems)
    mblk = mb_size * P_
    assert mblk == n_tile, "x chunk layout assumes mb block width == n_tile"
    k_step = 2
    perf = mybir.MatmulPerfMode.DoubleRow

    nc = bacc.Bacc("TRN2", target_bir_lowering=False, debug=False)
    if diag == "in_big":
        x5 = nc.dram_tensor(
            "x5", [mb_blocks, P_, kg_blocks * chunk_f], mybir.dt.float32,
            kind="ExternalInput",
        )
        w5 = nc.dram_tensor(
            "w5", [n_tiles, P_, kg_blocks * chunk_f], mybir.dt.float32,
            kind="ExternalInput",
        )
    else:
        x5 = nc.dram_tensor(
            "x5", [mb_blocks, kg_blocks, P_, chunk_f], mybir.dt.float32,
            kind="ExternalInput",
        )
        w5 = nc.dram_tensor(
            "w5", [n_tiles, kg_blocks, P_, chunk_f], mybir.dt.float32,
            kind="ExternalInput",
        )
    bias = nc.dram_tensor("bias", [P_, d_out_loc], mybir.dt.float32,
                          kind="ExternalInput")
    out = nc.dram_tensor("out", [t_loc, d_out_loc], mybir.dt.float32,
                         kind="ExternalOutput")

    ge = mybir.AluOpType.is_ge
    sub = mybir.AluOpType.subtract
    add = mybir.AluOpType.add

    with tile.TileContext(nc) as tc:
        with (
            tc.tile_pool(name="const", bufs=1) as const_pool,
            tc.tile_pool(name="xbin_pool", bufs=mb_blocks) as xbin_pool,
            tc.tile_pool(name="xstage", bufs=xstage_bufs) as xstage_pool,
            tc.tile_pool(name="wstage", bufs=wstage_bufs) as wstage_pool,
            tc.tile_pool(name="wbin", bufs=2) as wbin_pool,
            tc.tile_pool(name="evict", bufs=8) as evict_pool,
            tc.tile_pool(name="psum", bufs=8, space="PSUM") as psum_pool,
        ):
            bias_sb = const_pool.tile([P_, d_out_loc], mybir.dt.float32,
                                      name="bias_sb")
            nc.scalar.dma_start(bias_sb[:], bias[:])

            xbin_fixed = wbin_fixed = ob_fixed = None
            if diag == "no_in":
                xbin_fixed = const_pool.tile(
                    [P_, k_tiles, mblk], mm_dtype, name="xbin_fixed")
                for _k in range(k_tiles):
                    nc.any.memset(xbin_fixed[:, _k, :], 0.5)
                wbin_fixed = const_pool.tile(
                    [P_, k_tiles, n_tile], mm_dtype, name="wbin_fixed")
                nc.any.memset(wbin_fixed[:], 1.0)
                dummy_x = const_pool.tile([P_, 16], mybir.dt.float32,
                                          name="dummy_x")
                nc.sync.dma_start(dummy_x[:], x5[0, 0, :P_, :16])
                dummy_w = const_pool.tile([P_, 16], mybir.dt.float32,
                                          name="dummy_w")
                nc.sync.dma_start(dummy_w[:], w5[0, 0, :P_, :16])
            if diag in ("dma_only", "dma_sp", "in_only", "in_big"):
                ob_fixed = const_pool.tile([P_, n_tile], mybir.dt.float32,
                                           name="ob_fixed")
                nc.any.memset(ob_fixed[:], 1.0)
                if diag in ("in_only", "in_big"):
                    # keep the output referenced
                    nc.scalar.dma_start(out[:P_, :n_tile], ob_fixed[:])

            import contextlib

            rep_ctx = (
                tc.For_i(0, repeat, 1) if repeat > 1 else contextlib.nullcontext()
            )
            with rep_ctx:
                # --- W column n0 loads first, then x mb0..3, then W n1..3 ---
                wbins = [None] * n_tiles
                xbins = [None] * mb_blocks

                def load_w_col(n):
                    wbin = wbin_pool.tile([P_, k_tiles, n_tile], mm_dtype,
                                          name="wbin")
                    for kg in range(kg_blocks):
                        wf = wstage_pool.tile(
                            [P_, chunk_kt, n_tile], mybir.dt.float32, name="wf")
                        nc.sync.dma_start(wf[:], w5[n, kg])
                        nc.scalar.sign(
                            wbin[:, kg * chunk_kt : (kg + 1) * chunk_kt, :],
                            wf[:],
                        )
                    return wbin

                def load_x_blk(mb):
                    xbin = xbin_pool.tile([P_, k_tiles, mblk], mm_dtype,
                                          name="xbin")
                    for kg in range(kg_blocks):
                        xf = xstage_pool.tile(
                            [P_, chunk_kt, mblk], mybir.dt.float32, name="xf")
                        nc.sync.dma_start(xf[:], x5[mb, kg])
                        nc.vector.tensor_scalar(
                            xbin[:, kg * chunk_kt : (kg + 1) * chunk_kt, :],
                            xf[:], 0.0, 0.5, ge, sub,
                        )
                    return xbin

                if diag == "in_big":
                    # pure read stream: 8 MiB contiguous DMAs, no compute
                    for mb in range(mb_blocks):
                        xf = xstage_pool.tile(
                            [P_, kg_blocks * chunk_f], mybir.dt.float32,
                            name="xbig")
                        nc.sync.dma_start(xf[:], x5[mb])
                    for n in range(n_tiles):
                        wf = wstage_pool.tile(
                            [P_, kg_blocks * chunk_f], mybir.dt.float32,
                            name="wbig")
                        nc.sync.dma_start(wf[:], w5[n])
                elif diag != "no_in":
                    wbins[0] = load_w_col(0)
                    for mb in range(mb_blocks):
                        xbins[mb] = load_x_blk(mb)
                    for n in range(1, n_tiles):
                        wbins[n] = load_w_col(n)
                else:
                    wbins = [wbin_fixed] * n_tiles
                    xbins = [xbin_fixed] * mb_blocks

                if diag in ("dma_only", "dma_sp"):
                    eng = nc.scalar if diag == "dma_only" else nc.sync
                    for n in range(n_tiles):
                        for m in range(m_tiles):
                            eng.dma_start(
                                out[ts(m, P_), ts(n, n_tile)], ob_fixed[:])
                elif diag in ("in_only", "in_big"):
                    pass
                else:
                    for n in range(n_tiles):
                        wbin = wbins[n]
                        for mb in range(mb_blocks):
                            xbin = xbins[mb]
                            psums = [
                                psum_pool.tile([P_, n_tile], mybir.dt.float32,
                                               name="psum")
                                for _ in range(mb_size)
                            ]
                            for kb in range(kb_blocks):
                                for mi in range(mb_size):
                                    for kp in range(0, kb_tiles, k_step):
                                        k = kb * kb_tiles + kp
                                        nc.tensor.matmul(
                                            psums[mi][:],
                                            xbin[:, k : k + k_step, ts(mi, P_)],
                                            wbin[:, k : k + k_step, :],
                                            start=(kb == 0 and kp == 0),
                                            stop=(kb == kb_blocks - 1
                                                  and kp == kb_tiles - k_step),
                                            perf_mode=perf,
                                        )
                            for mi in range(mb_size):
                                m = mb * mb_size + mi
                                t1 = evict_pool.tile([P_, n_tile],
                                                     mybir.dt.float32, name="t1")
                                # psum = S/2 (x +-0.5, W +-1) -> exact 2x
                                nc.scalar.activation(
                                    t1[:], psums[mi][:],
                                    mybir.ActivationFunctionType.Copy, scale=2.0)
                                ob = evict_pool.tile([P_, n_tile],
                                                     mybir.dt.float32, name="ob")
                                nc.vector.tensor_tensor(
                                    ob[:], t1[:], bias_sb[:, ts(n, n_tile)], add)
                                nc.scalar.dma_start(
                                    out[ts(m, P_), ts(n, n_tile)], ob[:])

    nc.compile()
    return nc


def build_nc_v6(
    t_loc: int = 2048,
    d_in: int = D_IN,
    d_out_loc: int = 2048,
    n_tile: int = 512,
    chunk_kt: int = 4,
    mm_dtype: mybir.dt = mybir.dt.float8e4,
    repeat: int = 1,
    mb_size: int = 4,
    kb_tiles: int = 8,
    xstage_bufs: int = 2,
    wstage_bufs: int = 2,
    out_on_act: bool = True,
    out_dtype: mybir.dt = mybir.dt.float32,
    interleave0: bool = False,
    evict_bufs: int = 8,
    ham_keepalive: bool = False,
    gsub: int | None = None,
):
    """v6: v5 layouts + rectangle-growth delivery/emission order.

    Load order W0,X0,W1,X1,W2,X2,W3,X3 (8 MiB units, 1 MiB chunks); groups
    (mb,n) are emitted as the rectangle grows so the PE stream tracks DMA
    arrival:  [W0,X0]:(0,0)  W1:(0,1)  X1:(1,0),(1,1)  W2:(0,2),(1,2)
    X2:(2,*)  W3:(0..2,3)  X3:(3,*).
    All four W columns stay resident (wbin bufs=4).  Binarize on DVE for
    both x and W (exact is_ge: +-0.5, psum=S/4, ACT Copy scale=4 + DVE
    bias add).  Out writes + bias on the ACT HWDGE ring; loads on SP.
    """
    P_ = P
    assert t_loc % (mb_size * P_) == 0 and d_out_loc % n_tile == 0
    k_tiles = d_in // P_
    m_tiles = t_loc // P_
    n_tiles = d_out_loc // n_tile
    mb_blocks = m_tiles // mb_size
    assert k_tiles % chunk_kt == 0 and k_tiles % kb_tiles == 0
    kg_blocks = k_tiles // chunk_kt
    kb_blocks = k_tiles // kb_tiles
    chunk_f = chunk_kt * n_tile
    mblk = mb_size * P_
    assert mblk == n_tile
    k_step = 2
    perf = mybir.MatmulPerfMode.DoubleRow

    nc = bacc.Bacc("TRN2", target_bir_lowering=False, debug=False)
    x5 = nc.dram_tensor(
        "x5", [mb_blocks, kg_blocks, P_, chunk_f], mybir.dt.float32,
        kind="ExternalInput",
    )
    w5 = nc.dram_tensor(
        "w5", [n_tiles, kg_blocks, P_, chunk_f], mybir.dt.float32,
        kind="ExternalInput",
    )
    bias = nc.dram_tensor("bias", [P_, d_out_loc], mybir.dt.float32,
                          kind="ExternalInput")
    out = nc.dram_tensor("out", [t_loc, d_out_loc], out_dtype,
                         kind="ExternalOutput")

    ge = mybir.AluOpType.is_ge
    sub = mybir.AluOpType.subtract
    add = mybir.AluOpType.add
    out_eng = nc.scalar if out_on_act else nc.sync

    with tile.TileContext(nc) as tc:
        with (
            tc.tile_pool(name="const", bufs=1) as const_pool,
            tc.tile_pool(name="xbin_pool", bufs=mb_blocks) as xbin_pool,
            tc.tile_pool(name="xstage", bufs=xstage_bufs) as xstage_pool,
            tc.tile_pool(name="wstage", bufs=wstage_bufs) as wstage_pool,
            tc.tile_pool(name="wbin", bufs=n_tiles) as wbin_pool,
            tc.tile_pool(name="evict", bufs=evict_bufs) as evict_pool,
            tc.tile_pool(name="psum", bufs=7 if ham_keepalive else 8,
                         space="PSUM") as psum_pool,
        ):
            import contextlib as _ctxlib

            _stk = _ctxlib.ExitStack()
            psum_scratch_pool = (
                _stk.enter_context(
                    tc.tile_pool(name="psum_ka", bufs=1, space="PSUM"))
                if ham_keepalive else None
            )
            bias_sb = const_pool.tile([P_, d_out_loc], mybir.dt.float32,
                                      name="bias_sb")
            nc.scalar.dma_start(bias_sb[:], bias[:])

            import contextlib

            rep_ctx = (
                tc.For_i(0, repeat, 1) if repeat > 1 else contextlib.nullcontext()
            )
            with rep_ctx:
                wbins = [None] * n_tiles
                xbins = [None] * mb_blocks

                # Tiny matmul tied to a freshly binarized chunk: keeps the
                # PE HAM window from seeing a fully idle 3.4us stretch
                # during the DMA-paced phase (else it re-throttles to
                # 1.2 GHz and every burst restarts cold).
                ka_psum = None
                if ham_keepalive:
                    ka_psum = psum_scratch_pool.tile(
                        [P_, 64], mybir.dt.float32, name="ka_psum")

                def keepalive(bin_tile, kg):
                    if not ham_keepalive:
                        return
                    k0 = kg * chunk_kt
                    nc.tensor.matmul(
                        ka_psum[:],
                        bin_tile[:, k0 : k0 + 2, :P_],
                        bin_tile[:, k0 : k0 + 2, :64],
                        start=True, stop=True,
                        perf_mode=perf, skip_group_check=True,
                    )

                def load_w_col(n):
                    wbin = wbin_pool.tile([P_, k_tiles, n_tile], mm_dtype,
                                          name="wbin")
                    for kg in range(kg_blocks):
                        wf = wstage_pool.tile(
                            [P_, chunk_kt, n_tile], mybir.dt.float32, name="wf")
                        nc.sync.dma_start(wf[:], w5[n, kg])
                        nc.vector.tensor_scalar(
                            wbin[:, kg * chunk_kt : (kg + 1) * chunk_kt, :],
                            wf[:], 0.0, 0.5, ge, sub,
                        )
                        keepalive(wbin, kg)
                    wbins[n] = wbin

                def load_x_blk(mb):
                    xbin = xbin_pool.tile([P_, k_tiles, mblk], mm_dtype,
                                          name="xbin")
                    for kg in range(kg_blocks):
                        xf = xstage_pool.tile(
                            [P_, chunk_kt, mblk], mybir.dt.float32, name="xf")
                        nc.sync.dma_start(xf[:], x5[mb, kg])
                        nc.vector.tensor_scalar(
                            xbin[:, kg * chunk_kt : (kg + 1) * chunk_kt, :],
                            xf[:], 0.0, 0.5, ge, sub,
                        )
                        keepalive(xbin, kg)
                    xbins[mb] = xbin

                def group(mb, n):
                    g = gsub or mb_size
                    for sub in range(mb_size // g):
                        _subgroup(mb, n, sub * g, g)

                def _subgroup(mb, n, mi0, g):
                    xbin, wbin = xbins[mb], wbins[n]
                    psums = [
                        psum_pool.tile([P_, n_tile], mybir.dt.float32,
                                       name="psum")
                        for _ in range(g)
                    ]
                    for kb in range(kb_blocks):
                        for mi_l in range(g):
                            for kp in range(0, kb_tiles, k_step):
                                k = kb * kb_tiles + kp
                                nc.tensor.matmul(
                                    psums[mi_l][:],
                                    xbin[:, k : k + k_step,
                                         ts(mi0 + mi_l, P_)],
                                    wbin[:, k : k + k_step, :],
                                    start=(kb == 0 and kp == 0),
                                    stop=(kb == kb_blocks - 1
                                          and kp == kb_tiles - k_step),
                                    perf_mode=perf,
                                )
                    for mi_l in range(g):
                        mi = mi0 + mi_l
                        m = mb * mb_size + mi
                        t1 = evict_pool.tile([P_, n_tile], mybir.dt.float32,
                                             name="t1")
                        # psum = S/4 (both operands +-0.5) -> exact 4x
                        nc.scalar.activation(
                            t1[:], psums[mi_l][:],
                            mybir.ActivationFunctionType.Copy, scale=4.0)
                        ob = evict_pool.tile([P_, n_tile], out_dtype,
                                             name="ob")
                        nc.vector.tensor_tensor(
                            ob[:], t1[:], bias_sb[:, ts(n, n_tile)], add)
                        out_eng.dma_start(out[ts(m, P_), ts(n, n_tile)], ob[:])

                # rectangle growth: alternate W cols and x blocks.
                # Step 0 interleaves W0/X0 chunks so group (0,0) is k-paced
                # from the first ~2 MiB of arrivals.
                if interleave0:
                    wbin0 = wbin_pool.tile([P_, k_tiles, n_tile], mm_dtype,
                                           name="wbin")
                    xbin0 = xbin_pool.tile([P_, k_tiles, mblk], mm_dtype,
                                           name="xbin")
                    for kg in range(kg_blocks):
                        wf = wstage_pool.tile(
                            [P_, chunk_kt, n_tile], mybir.dt.float32, name="wf")
                        nc.sync.dma_start(wf[:], w5[0, kg])
                        xf = xstage_pool.tile(
                            [P_, chunk_kt, mblk], mybir.dt.float32, name="xf")
                        nc.sync.dma_start(xf[:], x5[0, kg])
                        nc.vector.tensor_scalar(
                            wbin0[:, kg * chunk_kt : (kg + 1) * chunk_kt, :],
                            wf[:], 0.0, 0.5, ge, sub,
                        )
                        keepalive(wbin0, kg)
                        nc.vector.tensor_scalar(
                            xbin0[:, kg * chunk_kt : (kg + 1) * chunk_kt, :],
                            xf[:], 0.0, 0.5, ge, sub,
                        )
                        keepalive(xbin0, kg)
                    wbins[0] = wbin0
                    xbins[0] = xbin0
                else:
                    load_w_col(0)
                    load_x_blk(0)
                group(0, 0)
                for s in range(1, n_tiles + mb_blocks - 1):
                    if s % 2 == 1:  # new W column
                        n = (s + 1) // 2
                        load_w_col(n)
                        for mb in range((s + 1) // 2):
                            group(mb, n)
                    else:  # new x block
                        mb = s // 2
                        load_x_blk(mb)
                        for n in range(s // 2 + 1):
                            group(mb, n)

            _stk.close()

    nc.compile()
    return nc


def make_in_maps_v5(x, fp_weight, fp_bias, chunk_kt: int = 4):
    """Host-side sharding + relayout (layout only: transpose/reshape/slice)."""
    xT = np.asarray(x, dtype=np.float32).T  # [D_IN, N_TOK]
    wT = np.asarray(fp_weight, dtype=np.float32).T  # [D_IN, D_OUT]
    bias = np.asarray(fp_bias, dtype=np.float32)
    kg_blocks = D_IN // (chunk_kt * P)
    in_maps = []
    for c in range(N_CORES):
        i, j = divmod(c, O_GRP)
        xs = xT[:, i * T_LOC : (i + 1) * T_LOC]  # [4096, 2048]
        ws = wT[:, j * O_LOC : (j + 1) * O_LOC]  # [4096, 2048]
        # [kg, kt, p, blk, c] -> [blk, kg, p, kt, c]
        x5 = np.ascontiguousarray(
            xs.reshape(kg_blocks, chunk_kt, P, 4, 512).transpose(3, 0, 2, 1, 4)
        ).reshape(4, kg_blocks, P, chunk_kt * 512)
        w5 = np.ascontiguousarray(
            ws.reshape(kg_blocks, chunk_kt, P, 4, 512).transpose(3, 0, 2, 1, 4)
        ).reshape(4, kg_blocks, P, chunk_kt * 512)
        in_maps.append(
            {
                "x5": x5,
                "w5": w5,
                "bias": np.ascontiguousarray(
                    np.broadcast_to(
                        bias[None, j * O_LOC : (j + 1) * O_LOC], (P, O_LOC)
                    )
                ),
            }
        )
    return in_maps


_NC_CACHE: dict = {}

# production sharding: 4-way tokens x 2-way out-features
T_GRP, O_GRP = 4, 2
T_LOC = N_TOK // T_GRP  # 2048
O_LOC = D_OUT // O_GRP  # 2048


# Production build: v6 rect-growth schedule, fp16 out on the ACT ring,
# interleaved step-0 loads, deep stage pipelining.  PROD_KW is shared by
# kernel() and test.py's repeat-loop timing builds.
PROD_CHUNK_KT = 2
PROD_KW = dict(
    chunk_kt=PROD_CHUNK_KT,
    xstage_bufs=4,
    wstage_bufs=4,
    evict_bufs=8,
    out_dtype=mybir.dt.float16,
    out_on_act=True,
    interleave0=True,
    ham_keepalive=False,
)


def build_production(repeat: int = 1):
    return build_nc_v6(repeat=repeat, **PROD_KW)


def make_in_maps_production(x, fp_weight, fp_bias):
    return make_in_maps_v5(x, fp_weight, fp_bias, chunk_kt=PROD_CHUNK_KT)


def _get_nc(key=("v6",)):
    if key not in _NC_CACHE:
        _NC_CACHE[key] = build_production()
    return _NC_CACHE[key]


def make_in_maps(x, fp_weight, fp_bias):
    """Host-side sharding (layout only: transpose + slice + replicate)."""
    xT = np.ascontiguousarray(np.asarray(x, dtype=np.float32).T)  # [D_IN, N_TOK]
    wT = np.ascontiguousarray(np.asarray(fp_weight, dtype=np.float32).T)
    bias = np.asarray(fp_bias, dtype=np.float32)
    in_maps = []
    for c in range(N_CORES):
        i, j = divmod(c, O_GRP)
        in_maps.append(
            {
                "xT": np.ascontiguousarray(xT[:, i * T_LOC : (i + 1) * T_LOC]),
                "wT": np.ascontiguousarray(wT[:, j * O_LOC : (j + 1) * O_LOC]),
                "bias": np.ascontiguousarray(
                    np.broadcast_to(
                        bias[None, j * O_LOC : (j + 1) * O_LOC], (P, O_LOC)
                    )
                ),
            }
        )
    return in_maps


def assemble(results) -> np.ndarray:
    out = np.empty((N_TOK, D_OUT), np.float32)
    for c in range(N_CORES):
        i, j = divmod(c, O_GRP)
        out[i * T_LOC : (i + 1) * T_LOC, j * O_LOC : (j + 1) * O_LOC] = results[c][
            "out"
        ]
    return out


def kernel(x: np.ndarray, fp_weight: np.ndarray, fp_bias: np.ndarray) -> np.ndarray:
    assert x.shape == (N_TOK, D_IN) and fp_weight.shape == (D_OUT, D_IN)
    from concourse.bass_utils import run_bass_kernel_spmd

    nc = _get_nc()
    in_maps = make_in_maps_production(x, fp_weight, fp_bias)
    res = run_bass_kernel_spmd(nc, in_maps, core_ids=list(range(N_CORES)))
    return assemble(res.results)

